# revision 44
# baseline (speedup 1.0000x reference)
"""Trainium2 Bass kernel for nn_PointTransformerLayer (N=1024, dim=64, 8 cores).

Sharding: query rows i are split across 8 cores (128 rows each, flash-attention
style); k/v/pos and all weights are replicated to every core host-side.

Math (per core, rows i in its slice, all j):
  a_i  = p_i @ W_pos1 + b_pos1            (per-i, precomputed)
  bn_j = -(p_j @ W_pos1)                  (per-j, precomputed)
  R    = relu(a_i + bn_j)                 -> bf16                    [pos MLP l1]
  U    = R @ W_pos2 - k_j + q_i + b_pos2  (k folded into the matmul via -I rows,
                                           q_i + b_pos2 added as evac bias)
  H    = relu(U @ W_attn1 + b_attn1)      -> bf16
  sim  = H @ W_attn2                      (b_attn2 dropped: softmax-invariant)
  E    = exp(sim)  (no max-sub: |sim| < ~1)
  agg  = (sum_j E*U + sum_j E*(v_j+k_j)) / sum_j E - q_i
       == softmax(sim) . (v_j + rpe)  since v_e = U + (v+k) - q

Layouts are feature-major: [features on partitions, points on free dim].
Two query rows are processed per iteration by packing their 64-wide feature
tensors into the 128 partitions (group A = rows 0..63, group B = rows 64..127
of the core's slice).

Optimizations vs the first working version:
  - den (sum_j E) comes free from the exp ACTIVATE's accum_out, so the DVE
    fold chain only processes the E*V product (P-only folds + FD-256 reduce).
  - U evac is a single FD-1024 ACTIVATE (2-bank PSUM read); sim/exp likewise
    merged (split_sim=False) to pay the accumulator-read cost once.
  - The elementwise tail (V add, E*V, folds, reduce) runs once per TWO
    iterations at FD=2048 to amortize per-op overhead (batch2).
  - All inputs arrive as two packed blobs (2 landing DMAs instead of 14).
  - H evacs are split ACT/DVE per h_act_pattern to balance the two engines.
(GpSimd offload and the fused scalar_tensor_tensor/tensor_tensor_reduce ops
were measured/found unusable on this runtime: 15us/op, 5.7us/op, and a
runtime failure respectively.)
"""

import sys

sys.path.insert(0, "/opt/trn_rl_repo")

import numpy as np
import ml_dtypes

import concourse.bass as bass
import concourse.bacc as bacc
import concourse.mybir as mybir
import concourse.tile as tile
from concourse.bass_utils import run_bass_kernel_spmd
from concourse.tile_rust import add_dep_helper

F32 = mybir.dt.float32
BF16 = mybir.dt.bfloat16
AF = mybir.ActivationFunctionType
OP = mybir.AluOpType

N = 1024
DIM = 64
HID = 256  # DIM * ATTN_MULT
NCORES = 8
ROWS = N // NCORES  # 128 query rows per core
NPAIR = ROWS // 2  # 64 iterations, 2 rows (A/B groups) each

_CACHE = {}
_CONFIG = {
    "r_pool": False,    # R tensor_scalar on GpSimd (measured 15us/op — keep off)
    "h_act_pattern": (2,),  # per-iteration count of H evacs on ACT
    "h_act": 2,         # fallback when h_act_pattern unset
    "split_sim": True,  # per-jc sim PSUM tiles + exp (better overlap)
    "batch2": True,     # run the elementwise tail once per TWO iterations
}


def _setup_phase(nc, tc, t):
    """Small precompute matmuls. Ordered so the tensors gating iteration 0
    (bn2/aA -> R, then Rbuf kT halves -> U) materialize first."""
    with tc.tile_pool(name="setup_ps", bufs=2, space="PSUM") as spool:
        # bn = -(p@Wpos1) over all j (bf16 both halves)   [gates R(0)]
        for jc in range(2):
            ps = spool.tile([DIM, 512], F32, tag="bt_ps", name="ps")
            nc.tensor.matmul(ps[:], t.Wpos1[:], t.pT[:, jc * 512:(jc + 1) * 512])
            nc.scalar.activation(
                t.bn2[0:DIM, jc * 512:(jc + 1) * 512], ps[:], AF.Identity,
                bias=0.0, scale=-1.0,
            )
        nc.vector.tensor_copy(t.bn2[DIM:128, :], t.bn2[0:DIM, :])
        # a-cols for this core's rows: a = p_i@Wpos1 + b_pos1  [gates R(0)]
        # stacked: partitions 0:64 = rows 0..63 (group A), 64:128 = group B
        aps = spool.tile([DIM, ROWS], F32, tag="a_ps", name="aps")
        nc.tensor.matmul(aps[:], t.Wpos1[:], t.pTs[:])
        nc.scalar.activation(
            t.aAB[0:DIM, :], aps[:, 0:NPAIR], AF.Identity, bias=t.bpos1[:]
        )
        nc.scalar.activation(
            t.aAB[DIM:128, :], aps[:, NPAIR:ROWS], AF.Identity, bias=t.bpos1[:]
        )
        # k^T, v^T over all j  (f32 matmuls)
        for jc in range(2):
            ps = spool.tile([DIM, 512], F32, tag="kv_ps", name="ps")
            nc.tensor.matmul(ps[:], t.Wk[:], t.xT[:, jc * 512:(jc + 1) * 512])
            nc.scalar.copy(t.tmp_kT[:, jc * 512:(jc + 1) * 512], ps[:])
            ps2 = spool.tile([DIM, 512], F32, tag="kv_ps", name="ps2")
            nc.tensor.matmul(ps2[:], t.Wv[:], t.xT[:, jc * 512:(jc + 1) * 512])
            nc.scalar.copy(t.tmp_vT[:, jc * 512:(jc + 1) * 512], ps2[:])
        # static k^T (bf16), duplicated to both partition halves [gates U(0)]
        nc.vector.tensor_copy(t.kT2[0:DIM, :], t.tmp_kT[:])
        nc.vector.tensor_copy(t.kT2[DIM:128, :], t.tmp_kT[:])
        # vk = v + k (bf16, both partition halves)
        nc.vector.tensor_tensor(
            out=t.vk2[0:DIM, :], in0=t.tmp_kT[:], in1=t.tmp_vT[:], op=OP.add
        )
        nc.vector.tensor_copy(t.vk2[DIM:128, :], t.vk2[0:DIM, :])
        if _CONFIG.get("batch2"):
            nc.vector.tensor_copy(t.vk4[:, 0:N], t.vk2[:])
            nc.vector.tensor_copy(t.vk4[:, N:2 * N], t.vk2[:])
        # q-cols for this core's rows
        qps = spool.tile([DIM, ROWS], F32, tag="q_ps", name="qps")
        nc.tensor.matmul(qps[:], t.Wq[:], t.xTs[:])
        nc.scalar.copy(t.qT2[0:DIM, :], qps[:, 0:NPAIR])
        nc.scalar.copy(t.qT2[DIM:128, :], qps[:, NPAIR:ROWS])
        nc.scalar.activation(t.qb2[0:DIM, :], qps[:, 0:NPAIR], AF.Identity, bias=t.bpos2[:])
        nc.scalar.activation(t.qb2[DIM:128, :], qps[:, NPAIR:ROWS], AF.Identity, bias=t.bpos2[:])


def _emit_R(nc, t, m):
    """R(m) = relu(a_m + bn_j) -> bf16, BOTH groups in one FD-1024 op
    (partitions 0:64 = group A, 64:128 = group B; bn2 is duplicated and the
    per-partition scalar column carries aA/aB stacked). Emitted `r_ahead`
    iterations early so the U matmuls never wait on it."""
    R2 = t.Rbufs[m % len(t.Rbufs)]
    nc.vector.tensor_scalar(
        out=R2[:], in0=t.bn2[:],
        scalar1=t.aAB[:, m:m + 1], scalar2=0.0, op0=OP.add, op1=OP.max,
    )


def _pair_iteration(nc, t, pools, m):
    """One iteration: two query rows (groups A/B) against all 1024 j."""
    wpool, hpool, upool, hpspool, simpool = pools
    R2 = t.Rbufs[m % len(t.Rbufs)]
    # U = Wpos2^T @ R - kT (+ q_i + b_pos2 as evac bias). Two accumulating
    # matmuls per group on the diagonal PE tiles (g*64, g*64); group chains
    # in one PSUM bank are serialized (has_written is cleared bank-wide by
    # each chain's start=True).
    half = m % 2 if _CONFIG.get("batch2") else 0
    if _CONFIG.get("batch2"):
        if half == 0:
            t.cur_U2 = wpool.tile([128, 2 * N], BF16, tag="U2", name="U2")
            t.cur_EP2 = wpool.tile([128, 2 * N], BF16, tag="EP2", name="EP2")
        U_sb = t.cur_U2[:, half * N:(half + 1) * N]
    else:
        U_sb = wpool.tile([128, N], BF16, tag="U_sb", name="U_sb")
    U_ps = upool.tile([128, N], F32, tag="U_ps", name="U_ps", bufs=1)
    for jc in range(2):
        sl = slice(jc * 512, (jc + 1) * 512)
        prev = None
        for g in range(2):
            gs = slice(g * DIM, (g + 1) * DIM)
            i1 = nc.tensor.matmul(
                U_ps[gs, sl], t.Wp2[gs, :], R2[gs, sl],
                start=True, stop=False, tile_position=(g * DIM, g * DIM),
            )
            i2 = nc.tensor.matmul(
                U_ps[gs, sl], t.negI[gs, :], t.kT2[gs, sl],
                start=False, stop=True, tile_position=(g * DIM, g * DIM),
            )
            if prev is not None:
                add_dep_helper(
                    i1.ins, prev.ins, False, "U psum zero-region chain order"
                )
            prev = i2
    nc.scalar.activation(
        U_sb[:], U_ps[:], AF.Identity, bias=t.qb2[:, m:m + 1]
    )
    # H = relu(U @ W_attn1 + b_attn1) -> bf16
    # one [128,1024] PSUM tile per (hb, jc): groups A/B in free-dim halves,
    # so the whole tile shares one per-partition bias (b_attn1[hb]) and the
    # evac is a single FD-1024 op, split between ACT and DVE per h_act.
    pat = _CONFIG.get("h_act_pattern")
    if pat:
        n_act = pat[m % len(pat)]
    else:
        h_act = _CONFIG.get("h_act", 2)
        n_act = int(h_act) + (1 if (h_act % 1) and (m % 2 == 1) else 0)
    H_sbs = {}
    evac_i = 0
    for hb in range(2):
        for jc in range(2):
            H_ps = hpspool.tile([128, 2 * 512], F32, tag="H_ps", name="H_ps")
            for g in range(2):
                nc.tensor.matmul(
                    H_ps[:, g * 512:(g + 1) * 512],
                    t.W1[g * DIM:(g + 1) * DIM, hb * 128:(hb + 1) * 128],
                    U_sb[g * DIM:(g + 1) * DIM, jc * 512:(jc + 1) * 512],
                    tile_position=(g * DIM, 0),
                )
            H_sb = hpool.tile([128, 2 * 512], BF16, tag="H_sb", name="H_sb")
            # interleave ACT/DVE evacs (ACT also has U/exp work)
            act_sets = {0: (), 1: (1,), 2: (0, 2), 3: (0, 2, 3), 4: (0, 1, 2, 3)}
            use_act = evac_i in act_sets[n_act]
            if use_act:
                nc.scalar.activation(
                    H_sb[:], H_ps[:], AF.Relu, bias=t.b1[:, hb:hb + 1]
                )
            else:
                nc.vector.tensor_scalar(
                    out=H_sb[:], in0=H_ps[:],
                    scalar1=t.b1[:, hb:hb + 1], scalar2=0.0,
                    op0=OP.add, op1=OP.max,
                )
            evac_i += 1
            H_sbs[(hb, jc)] = H_sb
    # sim = H @ W_attn2  (2-mm accumulation chains; keep each PSUM bank's
    # chains strictly sequential: group A completes before group B starts).
    # One 1-bank tile per jc half + per-jc exp (accum_out -> den) lets PE
    # run ahead of ACT.
    if _CONFIG.get("batch2"):
        EP = t.cur_EP2[:, half * N:(half + 1) * N]
    else:
        EP = wpool.tile([128, N], BF16, tag="EP", name="EP")
    if _CONFIG.get("split_sim", True):
        sim_tiles = [
            simpool.tile([128, 512], F32, tag="SIM_ps", name="SIM_ps")
            for _ in range(2)
        ]
    else:
        big = simpool.tile([128, N], F32, tag="SIM_ps", name="SIM_ps", bufs=1)
        sim_tiles = [big[:, 0:512], big[:, 512:1024]]
    for jc in range(2):
        SIM_ps = sim_tiles[jc]
        prev_last = None
        for g in range(2):
            insts = []
            for hb in range(2):
                inst = nc.tensor.matmul(
                    SIM_ps[g * DIM:(g + 1) * DIM, :],
                    t.W2[:, hb * DIM:(hb + 1) * DIM],
                    H_sbs[(hb, jc)][:, g * 512:(g + 1) * 512],
                    start=(hb == 0),
                    stop=(hb == 1),
                    tile_position=(0, g * DIM),
                )
                insts.append(inst)
            if prev_last is not None:
                add_dep_helper(
                    insts[0].ins, prev_last.ins, False,
                    "psum zero-region chain order",
                )
            prev_last = insts[1]
        if _CONFIG.get("split_sim", True):
            nc.scalar.activation(
                EP[:, jc * 512:(jc + 1) * 512], SIM_ps[:], AF.Exp,
                accum_out=t.dens[jc][:, m:m + 1],
            )
    if not _CONFIG.get("split_sim", True):
        nc.scalar.activation(
            EP[:, 0:N], big[:], AF.Exp, accum_out=t.dens[0][:, m:m + 1]
        )
    # v_e (mod q) = U + (v+k); P = E * V; then fold P pairwise at 2x and do
    # the final 1x tensor_reduce on only 256 elements per row. den comes from
    # the exp's accum_out, so the folds only process the product. With batch2
    # the whole tail runs once per TWO iterations at FD=2048.
    if _CONFIG.get("batch2"):
        if half == 1:
            V2 = wpool.tile([128, 2 * N], BF16, tag="V2", name="V2")
            nc.vector.tensor_tensor(
                out=V2[:], in0=t.cur_U2[:], in1=t.vk4[:], op=OP.add
            )
            SCR2 = wpool.tile([128, 2 * N], BF16, tag="SCR2", name="SCR2")
            nc.vector.tensor_tensor(
                out=SCR2[:], in0=t.cur_EP2[:], in1=V2[:], op=OP.mult
            )
            s3 = SCR2.rearrange("p (k n) -> p k n", k=2)
            F1 = wpool.tile([128, N], BF16, tag="F1", name="F1")
            f1 = F1.rearrange("p (k n) -> p k n", k=2)
            nc.vector.tensor_tensor(
                out=f1[:, :, :], in0=s3[:, :, 0:512], in1=s3[:, :, 512:1024],
                op=OP.add,
            )
            F2 = wpool.tile([128, N // 2], BF16, tag="F2", name="F2")
            f2 = F2.rearrange("p (k n) -> p k n", k=2)
            nc.vector.tensor_tensor(
                out=f2[:, :, :], in0=f1[:, :, 0:256], in1=f1[:, :, 256:512],
                op=OP.add,
            )
            F3 = wpool.tile([128, N // 4], BF16, tag="F3", name="F3")
            f3 = F3.rearrange("p (k n) -> p k n", k=2)
            nc.vector.tensor_tensor(
                out=f3[:, :, :], in0=f2[:, :, 0:128], in1=f2[:, :, 128:256],
                op=OP.add,
            )
            nc.vector.tensor_reduce(
                out=t.numU[:, m - 1:m + 1], in_=f3[:, :, :],
                axis=mybir.AxisListType.X, op=OP.add,
            )
    else:
        V_sb = wpool.tile([128, N], BF16, tag="V_sb", name="V_sb")
        nc.vector.tensor_tensor(out=V_sb[:], in0=U_sb[:], in1=t.vk2[:], op=OP.add)
        SCR1 = wpool.tile([128, N], BF16, tag="SCR1", name="SCR1")
        nc.vector.tensor_tensor(
            out=SCR1[:], in0=EP[:], in1=V_sb[:], op=OP.mult
        )
        F1 = wpool.tile([128, N // 2], BF16, tag="F1", name="F1")
        nc.vector.tensor_tensor(
            out=F1[:], in0=SCR1[:, 0:512], in1=SCR1[:, 512:1024], op=OP.add
        )
        F2 = wpool.tile([128, N // 4], BF16, tag="F2", name="F2")
        nc.vector.tensor_tensor(
            out=F2[:], in0=F1[:, 0:256], in1=F1[:, 256:512], op=OP.add
        )
        nc.vector.tensor_reduce(
            out=t.numU[:, m:m + 1], in_=F2[:],
            axis=mybir.AxisListType.X, op=OP.add,
        )


class _Tiles:
    pass


def _build_program(repeat=1):
    """Build the Bass program (same program for all 8 cores; per-core data
    comes from in_maps). Returns the Bass object. `repeat` re-runs the main
    loop N times inside the NEFF (for slope-based device timing)."""
    nc = bacc.Bacc("TRN2", debug=False, num_devices=1, target_bir_lowering=False)

    # ---- DRAM I/O ----
    # All inputs are packed host-side into two blobs (one per dtype) so the
    # kernel head issues 2 landing DMAs instead of 14.
    d_bP = nc.dram_tensor("blobP", [3, 1216], F32, kind="ExternalInput")
    d_bF = nc.dram_tensor("blobF", [128, 1348], F32, kind="ExternalInput")
    d_bB = nc.dram_tensor("blobB", [128, 512], BF16, kind="ExternalInput")
    d_out = nc.dram_tensor("agg_out", [128, NPAIR], F32, kind="ExternalOutput")

    with tile.TileContext(nc) as tc:
        with (
            tc.tile_pool(name="const", bufs=1) as cpool,
            tc.tile_pool(name="work", bufs=6) as wpool,
            tc.tile_pool(name="hsb", bufs=10) as hpool,
        ):
            t = _Tiles()
            # ---------------- persistent SBUF ----------------
            for name, shape, dt in (
                ("blobP", [3, 1216], F32),
                ("blobF", [128, 1348], F32), ("blobB", [128, 512], BF16),
                ("vk2", [128, N], BF16), ("vk4", [128, 2 * N], BF16),
                ("bn2", [128, N], BF16), ("kT2", [128, N], BF16),
                ("aAB", [128, NPAIR], F32),
                ("qT2", [128, NPAIR], F32), ("qb2", [128, NPAIR], F32),
                ("den0", [128, NPAIR], F32), ("den1", [128, NPAIR], F32),
                ("numU", [128, NPAIR], F32),
                ("tmp_kT", [DIM, N], F32), ("tmp_vT", [DIM, N], F32),
                ("warm", [128, 8], F32),
                ("recS0", [128, NPAIR], F32), ("agg", [128, NPAIR], F32),
            ):
                setattr(t, name, cpool.tile(shape, dt, tag=name, name=name))
            # blob slice views (same layout as _prep_inputs)
            bF, bB = t.blobF, t.blobB
            t.xT = bF[0:DIM, 0:1024]
            t.xTs = bF[0:DIM, 1024:1152]
            t.Wq = bF[0:DIM, 1152:1216]
            t.Wk = bF[0:DIM, 1216:1280]
            t.Wv = bF[0:DIM, 1280:1344]
            t.bpos1 = bF[0:DIM, 1344:1345]
            t.bpos2 = bF[0:DIM, 1345:1346]
            t.b1 = bF[:, 1346:1348]
            t.pT = t.blobP[:, 0:1024]
            t.pTs = t.blobP[:, 1024:1152]
            t.Wpos1 = t.blobP[:, 1152:1216]
            t.Wp2 = bB[:, 0:64]
            t.W1 = bB[:, 64:320]
            t.W2 = bB[:, 320:448]
            t.negI = bB[:, 448:512]
            t.dens = [t.den0, t.den1]
            t.Rbufs = [
                cpool.tile([128, N], BF16, tag=f"R2{p}", name=f"R2{p}")
                for p in range(3)
            ]

            # ---------------- DMA loads ----------------
            # the small pos blob lands first: it gates the bn/a matmuls that
            # feed R(0) and hence the whole pipeline
            nc.sync.dma_start(t.blobP[:], d_bP.ap())
            nc.sync.dma_start(t.blobF[:], d_bF.ap())
            nc.sync.dma_start(t.blobB[:], d_bB.ap())

            # preload the exp table set early (one-time ~2.7us)
            nc.gpsimd.memset(t.warm[:], 0.0)
            nc.scalar.activation(t.warm[:], t.warm[:], AF.Exp)

            _setup_phase(nc, tc, t)
            if not _CONFIG.get("split_sim", True):
                nc.vector.memset(t.den1[:], 0.0)

            # ---------------- main loop over row pairs ----------------
            with (
                tc.tile_pool(name="u_ps", bufs=2, space="PSUM") as upool,
                tc.tile_pool(name="h_ps", bufs=2, space="PSUM") as hpspool,
                tc.tile_pool(name="s_ps", bufs=2, space="PSUM") as simpool,
            ):
                pools = (wpool, hpool, upool, hpspool, simpool)
                r_ahead = _CONFIG.get("r_ahead", 2)
                for _r in range(repeat):
                    for m in range(min(r_ahead, NPAIR)):
                        _emit_R(nc, t, m)
                    for m in range(NPAIR):
                        if m + r_ahead < NPAIR:
                            _emit_R(nc, t, m + r_ahead)
                        _pair_iteration(nc, t, pools, m)

            # ---------------- finalize ----------------
            nc.vector.tensor_tensor(
                out=t.den0[:], in0=t.den0[:], in1=t.den1[:], op=OP.add
            )
            nc.vector.reciprocal(t.recS0[:], t.den0[:])
            nc.vector.tensor_tensor(
                out=t.agg[:], in0=t.numU[:], in1=t.recS0[:], op=OP.mult
            )
            nc.vector.tensor_tensor(
                out=t.agg[:], in0=t.agg[:], in1=t.qT2[:], op=OP.subtract
            )
            nc.sync.dma_start(d_out.ap(), t.agg[:])

    nc.compile()
    return nc


def _prep_inputs(x, pos, W_qkv, W_pos1, b_pos1, W_pos2, b_pos2,
                 W_attn1, b_attn1, W_attn2, b_attn2):
    """Host-side data prep: slicing/transposes/weight packing (no O(N^2) math)."""
    bf = ml_dtypes.bfloat16
    x2 = np.ascontiguousarray(np.asarray(x, np.float32).reshape(N, DIM))
    p2 = np.ascontiguousarray(np.asarray(pos, np.float32).reshape(N, 3))
    xT = np.ascontiguousarray(x2.T)  # (64, N)
    pT = np.ascontiguousarray(p2.T)  # (3, N)
    W_qkv = np.asarray(W_qkv, np.float32)
    Wq = np.ascontiguousarray(W_qkv[:, 0:DIM])
    Wk = np.ascontiguousarray(W_qkv[:, DIM:2 * DIM])
    Wv = np.ascontiguousarray(W_qkv[:, 2 * DIM:3 * DIM])
    Wp2d = np.concatenate(
        [np.asarray(W_pos2, np.float32)] * 2, axis=0
    ).astype(bf)  # (128, 64): Wpos2 for both group row-blocks
    negI2 = np.concatenate(
        [-np.eye(DIM, dtype=np.float32)] * 2, axis=0
    ).astype(bf)  # (128, 64)
    W1dup = np.concatenate(
        [np.asarray(W_attn1, np.float32)] * 2, axis=0
    ).astype(bf)  # (128, 256)
    W2c = np.asarray(W_attn2, np.float32)
    W2cat = np.concatenate([W2c[0:128, :], W2c[128:256, :]], axis=1).astype(bf)
    b1c = np.ascontiguousarray(
        np.asarray(b_attn1, np.float32).reshape(2, 128).T
    )  # (128, 2)
    blobB = np.zeros((128, 512), dtype=bf)
    blobB[:, 0:64] = Wp2d
    blobB[:, 64:320] = W1dup
    blobB[:, 320:448] = W2cat
    blobB[:, 448:512] = negI2

    blobF = np.zeros((128, 1348), dtype=np.float32)
    blobF[0:DIM, 0:1024] = xT
    blobF[0:DIM, 1152:1216] = Wq
    blobF[0:DIM, 1216:1280] = Wk
    blobF[0:DIM, 1280:1344] = Wv
    blobF[0:DIM, 1344:1345] = np.asarray(b_pos1, np.float32).reshape(DIM, 1)
    blobF[0:DIM, 1345:1346] = np.asarray(b_pos2, np.float32).reshape(DIM, 1)
    blobF[:, 1346:1348] = b1c

    in_maps = []
    for c in range(NCORES):
        bFc = blobF.copy()
        bFc[0:DIM, 1024:1152] = xT[:, c * ROWS:(c + 1) * ROWS]
        bPc = np.zeros((3, 1216), np.float32)
        bPc[:, 0:1024] = pT
        bPc[:, 1024:1152] = pT[:, c * ROWS:(c + 1) * ROWS]
        bPc[:, 1152:1216] = np.asarray(W_pos1, np.float32)
        in_maps.append({"blobP": bPc, "blobF": bFc, "blobB": blobB})
    return in_maps


def kernel(x, pos, W_qkv, W_pos1, b_pos1, W_pos2, b_pos2,
           W_attn1, b_attn1, W_attn2, b_attn2, _want_trace=False):
    if "nc" not in _CACHE:
        _CACHE["nc"] = _build_program()
    nc = _CACHE["nc"]
    in_maps = _prep_inputs(x, pos, W_qkv, W_pos1, b_pos1, W_pos2, b_pos2,
                           W_attn1, b_attn1, W_attn2, b_attn2)
    res = run_bass_kernel_spmd(
        nc, in_maps, core_ids=list(range(NCORES)), trace=_want_trace
    )
    _CACHE["last_results"] = res
    out = np.empty((N, DIM), np.float32)
    for c in range(NCORES):
        agg = np.asarray(res.results[c]["agg_out"], np.float32)  # (128, 64)
        out[c * ROWS:c * ROWS + NPAIR, :] = agg[0:DIM, :].T
        out[c * ROWS + NPAIR:(c + 1) * ROWS, :] = agg[DIM:128, :].T
    return out.reshape(1, N, DIM)


# revision 53
# speedup vs baseline: 1.0864x; 1.0864x over previous
"""Trainium2 Bass kernel for nn_PointTransformerLayer (N=1024, dim=64, 8 cores).

Sharding: query rows i are split across 8 cores (128 rows each, flash-attention
style); k/v/pos and all weights are replicated to every core host-side.

Math (per core, rows i in its slice, all j):
  a_i  = p_i @ W_pos1 + b_pos1            (per-i, precomputed)
  bn_j = -(p_j @ W_pos1)                  (per-j, precomputed)
  R    = relu(a_i + bn_j)                 -> bf16                    [pos MLP l1]
  U    = R @ W_pos2 - k_j + q_i + b_pos2  (k folded into the matmul via -I rows,
                                           q_i + b_pos2 added as evac bias)
  H    = relu(U @ W_attn1 + b_attn1)      -> bf16
  sim  = H @ W_attn2                      (b_attn2 dropped: softmax-invariant)
  E    = exp(sim)  (no max-sub: |sim| < ~1)
  agg  = (sum_j E*U + sum_j E*(v_j+k_j)) / sum_j E - q_i
       == softmax(sim) . (v_j + rpe)  since v_e = U + (v+k) - q

Layouts are feature-major: [features on partitions, points on free dim].
Two query rows are processed per iteration by packing their 64-wide feature
tensors into the 128 partitions (group A = rows 0..63, group B = rows 64..127
of the core's slice).

Optimizations vs the first working version:
  - den (sum_j E) comes free from the exp ACTIVATE's accum_out, so the DVE
    fold chain only processes the E*V product (P-only folds + FD-256 reduce).
  - U evac is a single FD-1024 ACTIVATE (2-bank PSUM read); sim/exp likewise
    merged (split_sim=False) to pay the accumulator-read cost once.
  - The elementwise tail (V add, E*V, folds, reduce) runs once per TWO
    iterations at FD=2048 to amortize per-op overhead (batch2).
  - All inputs arrive as two packed blobs (2 landing DMAs instead of 14).
  - H evacs are split ACT/DVE per h_act_pattern to balance the two engines.
(GpSimd offload and the fused scalar_tensor_tensor/tensor_tensor_reduce ops
were measured/found unusable on this runtime: 15us/op, 5.7us/op, and a
runtime failure respectively.)
"""

import sys

sys.path.insert(0, "/opt/trn_rl_repo")

import numpy as np
import ml_dtypes

import concourse.bass as bass
import concourse.bacc as bacc
import concourse.mybir as mybir
import concourse.tile as tile
from concourse.bass_utils import run_bass_kernel_spmd
from concourse.tile_rust import add_dep_helper

F32 = mybir.dt.float32
BF16 = mybir.dt.bfloat16
AF = mybir.ActivationFunctionType
OP = mybir.AluOpType

N = 1024
DIM = 64
HID = 256  # DIM * ATTN_MULT
NCORES = 8
ROWS = N // NCORES  # 128 query rows per core
NPAIR = ROWS // 2  # 64 iterations, 2 rows (A/B groups) each

_CACHE = {}
_CONFIG = {
    "r_pool": False,    # R tensor_scalar on GpSimd (measured 15us/op — keep off)
    "h_act_pattern": (2, 2, 2, 3),  # per-iteration count of H evacs on ACT
    "h_act": 2,         # fallback when h_act_pattern unset
    "split_sim": True,  # per-jc sim PSUM tiles + exp (better overlap)
    "batch2": True,     # run the elementwise tail once per TWO iterations
}


def _setup_phase(nc, tc, t):
    """Small precompute matmuls. Ordered so the tensors gating iteration 0
    (bn2/aA -> R, then Rbuf kT halves -> U) materialize first."""
    with tc.tile_pool(name="setup_ps", bufs=2, space="PSUM") as spool:
        # bn = -(p@Wpos1) over all j (bf16 both halves)   [gates R(0)]
        for jc in range(2):
            ps = spool.tile([DIM, 512], F32, tag="bt_ps", name="ps")
            nc.tensor.matmul(ps[:], t.Wpos1[:], t.pT[:, jc * 512:(jc + 1) * 512])
            nc.scalar.activation(
                t.bn2[0:DIM, jc * 512:(jc + 1) * 512], ps[:], AF.Identity,
                bias=0.0, scale=-1.0,
            )
        nc.vector.tensor_copy(t.bn2[DIM:128, :], t.bn2[0:DIM, :])
        # a-cols for this core's rows: a = p_i@Wpos1 + b_pos1  [gates R(0)]
        # stacked: partitions 0:64 = rows 0..63 (group A), 64:128 = group B
        aps = spool.tile([DIM, ROWS], F32, tag="a_ps", name="aps")
        nc.tensor.matmul(aps[:], t.Wpos1[:], t.pTs[:])
        nc.scalar.activation(
            t.aAB[0:DIM, :], aps[:, 0:NPAIR], AF.Identity, bias=t.bpos1[:]
        )
        nc.scalar.activation(
            t.aAB[DIM:128, :], aps[:, NPAIR:ROWS], AF.Identity, bias=t.bpos1[:]
        )
        # k^T, v^T over all j  (f32 matmuls)
        for jc in range(2):
            ps = spool.tile([DIM, 512], F32, tag="kv_ps", name="ps")
            nc.tensor.matmul(ps[:], t.Wk[:], t.xT[:, jc * 512:(jc + 1) * 512])
            nc.scalar.copy(t.tmp_kT[:, jc * 512:(jc + 1) * 512], ps[:])
            ps2 = spool.tile([DIM, 512], F32, tag="kv_ps", name="ps2")
            nc.tensor.matmul(ps2[:], t.Wv[:], t.xT[:, jc * 512:(jc + 1) * 512])
            nc.scalar.copy(t.tmp_vT[:, jc * 512:(jc + 1) * 512], ps2[:])
        # static k^T (bf16) into partitions 64:128 of the R buffers [gates U(0)]
        for RA, RB in t.Rbufs:
            nc.vector.tensor_copy(RA[DIM:128, :], t.tmp_kT[:])
            nc.vector.tensor_copy(RB[DIM:128, :], t.tmp_kT[:])
        # vk = v + k (bf16, both partition halves)
        nc.vector.tensor_tensor(
            out=t.vk2[0:DIM, :], in0=t.tmp_kT[:], in1=t.tmp_vT[:], op=OP.add
        )
        nc.vector.tensor_copy(t.vk2[DIM:128, :], t.vk2[0:DIM, :])
        if _CONFIG.get("batch2"):
            nc.vector.tensor_copy(t.vk4[:, 0:N], t.vk2[:])
            nc.vector.tensor_copy(t.vk4[:, N:2 * N], t.vk2[:])
        # q-cols for this core's rows
        qps = spool.tile([DIM, ROWS], F32, tag="q_ps", name="qps")
        nc.tensor.matmul(qps[:], t.Wq[:], t.xTs[:])
        nc.scalar.copy(t.qT2[0:DIM, :], qps[:, 0:NPAIR])
        nc.scalar.copy(t.qT2[DIM:128, :], qps[:, NPAIR:ROWS])
        nc.scalar.activation(t.qb2[0:DIM, :], qps[:, 0:NPAIR], AF.Identity, bias=t.bpos2[:])
        nc.scalar.activation(t.qb2[DIM:128, :], qps[:, NPAIR:ROWS], AF.Identity, bias=t.bpos2[:])


def _emit_R(nc, t, m):
    """R(m) = relu(a_m + bn_j) -> bf16 into the rotating R buffers (kT lives
    statically in partitions 64:128). Emitted `r_ahead` iterations early so
    the U matmul never waits on it."""
    RA, RB = t.Rbufs[m % len(t.Rbufs)]
    nc.vector.tensor_scalar(
        out=RA[0:DIM, :], in0=t.bn2[0:DIM, :],
        scalar1=t.aAB[0:DIM, m:m + 1], scalar2=0.0, op0=OP.add, op1=OP.max,
    )
    nc.vector.tensor_scalar(
        out=RB[0:DIM, :], in0=t.bn2[0:DIM, :],
        scalar1=t.aAB[DIM:128, m:m + 1], scalar2=0.0, op0=OP.add, op1=OP.max,
    )


def _pair_iteration(nc, t, pools, m):
    """One iteration: two query rows (groups A/B) against all 1024 j."""
    wpool, hpool, upool, hpspool, simpool = pools
    RA, RB = t.Rbufs[m % len(t.Rbufs)]
    # U = [Wpos2; -I]^T @ [R; kT]  (single-mm groups, col-tiled concurrent)
    half = m % 2 if _CONFIG.get("batch2") else 0
    if _CONFIG.get("batch2"):
        if half == 0:
            t.cur_U2 = wpool.tile([128, 2 * N], BF16, tag="U2", name="U2")
            t.cur_EP2 = wpool.tile([128, 2 * N], BF16, tag="EP2", name="EP2")
        U_sb = t.cur_U2[:, half * N:(half + 1) * N]
    else:
        U_sb = wpool.tile([128, N], BF16, tag="U_sb", name="U_sb")
    U_ps = upool.tile([128, N], F32, tag="U_ps", name="U_ps", bufs=1)
    for jc in range(2):
        for g, Rt in ((0, RA), (1, RB)):
            nc.tensor.matmul(
                U_ps[g * DIM:(g + 1) * DIM, jc * 512:(jc + 1) * 512],
                t.Wp2I[:],
                Rt[:, jc * 512:(jc + 1) * 512],
                tile_position=(0, g * DIM),
            )
    nc.scalar.activation(
        U_sb[:], U_ps[:], AF.Identity, bias=t.qb2[:, m:m + 1]
    )
    # H = relu(U @ W_attn1 + b_attn1) -> bf16
    # one [128,1024] PSUM tile per (hb, jc): groups A/B in free-dim halves,
    # so the whole tile shares one per-partition bias (b_attn1[hb]) and the
    # evac is a single FD-1024 op, split between ACT and DVE per h_act.
    pat = _CONFIG.get("h_act_pattern")
    if pat:
        n_act = pat[m % len(pat)]
    else:
        h_act = _CONFIG.get("h_act", 2)
        n_act = int(h_act) + (1 if (h_act % 1) and (m % 2 == 1) else 0)
    H_sbs = {}
    evac_i = 0
    for hb in range(2):
        for jc in range(2):
            H_ps = hpspool.tile([128, 2 * 512], F32, tag="H_ps", name="H_ps")
            for g in range(2):
                nc.tensor.matmul(
                    H_ps[:, g * 512:(g + 1) * 512],
                    t.W1[g * DIM:(g + 1) * DIM, hb * 128:(hb + 1) * 128],
                    U_sb[g * DIM:(g + 1) * DIM, jc * 512:(jc + 1) * 512],
                    tile_position=(g * DIM, 0),
                )
            H_sb = hpool.tile([128, 2 * 512], BF16, tag="H_sb", name="H_sb")
            # interleave ACT/DVE evacs (ACT also has U/exp work)
            act_sets = {0: (), 1: (1,), 2: (0, 2), 3: (0, 2, 3), 4: (0, 1, 2, 3)}
            use_act = evac_i in act_sets[n_act]
            if use_act:
                nc.scalar.activation(
                    H_sb[:], H_ps[:], AF.Relu, bias=t.b1[:, hb:hb + 1]
                )
            else:
                nc.vector.tensor_scalar(
                    out=H_sb[:], in0=H_ps[:],
                    scalar1=t.b1[:, hb:hb + 1], scalar2=0.0,
                    op0=OP.add, op1=OP.max,
                )
            evac_i += 1
            H_sbs[(hb, jc)] = H_sb
    # sim = H @ W_attn2  (2-mm accumulation chains; keep each PSUM bank's
    # chains strictly sequential: group A completes before group B starts).
    # One 1-bank tile per jc half + per-jc exp (accum_out -> den) lets PE
    # run ahead of ACT.
    if _CONFIG.get("batch2"):
        EP = t.cur_EP2[:, half * N:(half + 1) * N]
    else:
        EP = wpool.tile([128, N], BF16, tag="EP", name="EP")
    if _CONFIG.get("split_sim", True):
        sim_tiles = [
            simpool.tile([128, 512], F32, tag="SIM_ps", name="SIM_ps")
            for _ in range(2)
        ]
    else:
        big = simpool.tile([128, N], F32, tag="SIM_ps", name="SIM_ps", bufs=1)
        sim_tiles = [big[:, 0:512], big[:, 512:1024]]
    for jc in range(2):
        SIM_ps = sim_tiles[jc]
        prev_last = None
        for g in range(2):
            insts = []
            for hb in range(2):
                inst = nc.tensor.matmul(
                    SIM_ps[g * DIM:(g + 1) * DIM, :],
                    t.W2[:, hb * DIM:(hb + 1) * DIM],
                    H_sbs[(hb, jc)][:, g * 512:(g + 1) * 512],
                    start=(hb == 0),
                    stop=(hb == 1),
                    tile_position=(0, g * DIM),
                )
                insts.append(inst)
            if prev_last is not None:
                add_dep_helper(
                    insts[0].ins, prev_last.ins, False,
                    "psum zero-region chain order",
                )
            prev_last = insts[1]
        if _CONFIG.get("split_sim", True):
            nc.scalar.activation(
                EP[:, jc * 512:(jc + 1) * 512], SIM_ps[:], AF.Exp,
                accum_out=t.dens[jc][:, m:m + 1],
            )
    if not _CONFIG.get("split_sim", True):
        nc.scalar.activation(
            EP[:, 0:N], big[:], AF.Exp, accum_out=t.dens[0][:, m:m + 1]
        )
    # v_e (mod q) = U + (v+k); P = E * V; then fold P pairwise at 2x and do
    # the final 1x tensor_reduce on only 256 elements per row. den comes from
    # the exp's accum_out, so the folds only process the product. With batch2
    # the whole tail runs once per TWO iterations at FD=2048.
    if _CONFIG.get("batch2"):
        if half == 1:
            V2 = wpool.tile([128, 2 * N], BF16, tag="V2", name="V2")
            nc.vector.tensor_tensor(
                out=V2[:], in0=t.cur_U2[:], in1=t.vk4[:], op=OP.add
            )
            SCR2 = wpool.tile([128, 2 * N], BF16, tag="SCR2", name="SCR2")
            nc.vector.tensor_tensor(
                out=SCR2[:], in0=t.cur_EP2[:], in1=V2[:], op=OP.mult
            )
            s3 = SCR2.rearrange("p (k n) -> p k n", k=2)
            F1 = wpool.tile([128, N], BF16, tag="F1", name="F1")
            f1 = F1.rearrange("p (k n) -> p k n", k=2)
            nc.vector.tensor_tensor(
                out=f1[:, :, :], in0=s3[:, :, 0:512], in1=s3[:, :, 512:1024],
                op=OP.add,
            )
            F2 = wpool.tile([128, N // 2], BF16, tag="F2", name="F2")
            f2 = F2.rearrange("p (k n) -> p k n", k=2)
            nc.vector.tensor_tensor(
                out=f2[:, :, :], in0=f1[:, :, 0:256], in1=f1[:, :, 256:512],
                op=OP.add,
            )
            F3 = wpool.tile([128, N // 4], BF16, tag="F3", name="F3")
            f3 = F3.rearrange("p (k n) -> p k n", k=2)
            nc.vector.tensor_tensor(
                out=f3[:, :, :], in0=f2[:, :, 0:128], in1=f2[:, :, 128:256],
                op=OP.add,
            )
            nc.vector.tensor_reduce(
                out=t.numU[:, m - 1:m + 1], in_=f3[:, :, :],
                axis=mybir.AxisListType.X, op=OP.add,
            )
    else:
        V_sb = wpool.tile([128, N], BF16, tag="V_sb", name="V_sb")
        nc.vector.tensor_tensor(out=V_sb[:], in0=U_sb[:], in1=t.vk2[:], op=OP.add)
        SCR1 = wpool.tile([128, N], BF16, tag="SCR1", name="SCR1")
        nc.vector.tensor_tensor(
            out=SCR1[:], in0=EP[:], in1=V_sb[:], op=OP.mult
        )
        F1 = wpool.tile([128, N // 2], BF16, tag="F1", name="F1")
        nc.vector.tensor_tensor(
            out=F1[:], in0=SCR1[:, 0:512], in1=SCR1[:, 512:1024], op=OP.add
        )
        F2 = wpool.tile([128, N // 4], BF16, tag="F2", name="F2")
        nc.vector.tensor_tensor(
            out=F2[:], in0=F1[:, 0:256], in1=F1[:, 256:512], op=OP.add
        )
        nc.vector.tensor_reduce(
            out=t.numU[:, m:m + 1], in_=F2[:],
            axis=mybir.AxisListType.X, op=OP.add,
        )


class _Tiles:
    pass


def _build_program(repeat=1):
    """Build the Bass program (same program for all 8 cores; per-core data
    comes from in_maps). Returns the Bass object. `repeat` re-runs the main
    loop N times inside the NEFF (for slope-based device timing)."""
    nc = bacc.Bacc("TRN2", debug=False, num_devices=1, target_bir_lowering=False)

    # ---- DRAM I/O ----
    # All inputs are packed host-side into two blobs (one per dtype) so the
    # kernel head issues 2 landing DMAs instead of 14.
    d_bP = nc.dram_tensor("blobP", [3, 1216], F32, kind="ExternalInput")
    d_bF = nc.dram_tensor("blobF", [128, 1348], F32, kind="ExternalInput")
    d_bB = nc.dram_tensor("blobB", [128, 512], BF16, kind="ExternalInput")
    d_out = nc.dram_tensor("agg_out", [128, NPAIR], F32, kind="ExternalOutput")

    with tile.TileContext(nc) as tc:
        with (
            tc.tile_pool(name="const", bufs=1) as cpool,
            tc.tile_pool(name="work", bufs=6) as wpool,
            tc.tile_pool(name="hsb", bufs=10) as hpool,
        ):
            t = _Tiles()
            # ---------------- persistent SBUF ----------------
            for name, shape, dt in (
                ("blobP", [3, 1216], F32),
                ("blobF", [128, 1348], F32), ("blobB", [128, 512], BF16),
                ("vk2", [128, N], BF16), ("vk4", [128, 2 * N], BF16),
                ("bn2", [128, N], BF16),
                ("aAB", [128, NPAIR], F32),
                ("qT2", [128, NPAIR], F32), ("qb2", [128, NPAIR], F32),
                ("den0", [128, NPAIR], F32), ("den1", [128, NPAIR], F32),
                ("numU", [128, NPAIR], F32),
                ("tmp_kT", [DIM, N], F32), ("tmp_vT", [DIM, N], F32),
                ("warm", [128, 8], F32),
                ("recS0", [128, NPAIR], F32), ("agg", [128, NPAIR], F32),
            ):
                setattr(t, name, cpool.tile(shape, dt, tag=name, name=name))
            # blob slice views (same layout as _prep_inputs)
            bF, bB = t.blobF, t.blobB
            t.xT = bF[0:DIM, 0:1024]
            t.xTs = bF[0:DIM, 1024:1152]
            t.Wq = bF[0:DIM, 1152:1216]
            t.Wk = bF[0:DIM, 1216:1280]
            t.Wv = bF[0:DIM, 1280:1344]
            t.bpos1 = bF[0:DIM, 1344:1345]
            t.bpos2 = bF[0:DIM, 1345:1346]
            t.b1 = bF[:, 1346:1348]
            t.pT = t.blobP[:, 0:1024]
            t.pTs = t.blobP[:, 1024:1152]
            t.Wpos1 = t.blobP[:, 1152:1216]
            t.Wp2I = bB[:, 0:64]
            t.W1 = bB[:, 64:320]
            t.W2 = bB[:, 320:448]
            t.dens = [t.den0, t.den1]
            t.Rbufs = [
                (cpool.tile([128, N], BF16, tag=f"RA{p}", name=f"RA{p}"),
                 cpool.tile([128, N], BF16, tag=f"RB{p}", name=f"RB{p}"))
                for p in range(3)
            ]

            # ---------------- DMA loads ----------------
            # the small pos blob lands first: it gates the bn/a matmuls that
            # feed R(0) and hence the whole pipeline
            nc.sync.dma_start(t.blobP[:], d_bP.ap())
            nc.sync.dma_start(t.blobF[:], d_bF.ap())
            nc.sync.dma_start(t.blobB[:], d_bB.ap())

            # preload the exp table set early (one-time ~2.7us)
            nc.gpsimd.memset(t.warm[:], 0.0)
            nc.scalar.activation(t.warm[:], t.warm[:], AF.Exp)

            _setup_phase(nc, tc, t)
            if not _CONFIG.get("split_sim", True):
                nc.vector.memset(t.den1[:], 0.0)

            # ---------------- main loop over row pairs ----------------
            with (
                tc.tile_pool(name="u_ps", bufs=2, space="PSUM") as upool,
                tc.tile_pool(name="h_ps", bufs=2, space="PSUM") as hpspool,
                tc.tile_pool(name="s_ps", bufs=2, space="PSUM") as simpool,
            ):
                pools = (wpool, hpool, upool, hpspool, simpool)
                r_ahead = _CONFIG.get("r_ahead", 2)
                for _r in range(repeat):
                    for m in range(min(r_ahead, NPAIR)):
                        _emit_R(nc, t, m)
                    for m in range(NPAIR):
                        if m + r_ahead < NPAIR:
                            _emit_R(nc, t, m + r_ahead)
                        _pair_iteration(nc, t, pools, m)

            # ---------------- finalize ----------------
            nc.vector.tensor_tensor(
                out=t.den0[:], in0=t.den0[:], in1=t.den1[:], op=OP.add
            )
            nc.vector.reciprocal(t.recS0[:], t.den0[:])
            nc.vector.tensor_tensor(
                out=t.agg[:], in0=t.numU[:], in1=t.recS0[:], op=OP.mult
            )
            nc.vector.tensor_tensor(
                out=t.agg[:], in0=t.agg[:], in1=t.qT2[:], op=OP.subtract
            )
            nc.sync.dma_start(d_out.ap(), t.agg[:])

    nc.compile()
    return nc


def _prep_inputs(x, pos, W_qkv, W_pos1, b_pos1, W_pos2, b_pos2,
                 W_attn1, b_attn1, W_attn2, b_attn2):
    """Host-side data prep: slicing/transposes/weight packing (no O(N^2) math)."""
    bf = ml_dtypes.bfloat16
    x2 = np.ascontiguousarray(np.asarray(x, np.float32).reshape(N, DIM))
    p2 = np.ascontiguousarray(np.asarray(pos, np.float32).reshape(N, 3))
    xT = np.ascontiguousarray(x2.T)  # (64, N)
    pT = np.ascontiguousarray(p2.T)  # (3, N)
    W_qkv = np.asarray(W_qkv, np.float32)
    Wq = np.ascontiguousarray(W_qkv[:, 0:DIM])
    Wk = np.ascontiguousarray(W_qkv[:, DIM:2 * DIM])
    Wv = np.ascontiguousarray(W_qkv[:, 2 * DIM:3 * DIM])
    Wp2I = np.concatenate(
        [np.asarray(W_pos2, np.float32), -np.eye(DIM, dtype=np.float32)], axis=0
    ).astype(bf)  # (128, 64): [Wpos2; -I] for the combined U matmul
    W1dup = np.concatenate(
        [np.asarray(W_attn1, np.float32)] * 2, axis=0
    ).astype(bf)  # (128, 256)
    W2c = np.asarray(W_attn2, np.float32)
    W2cat = np.concatenate([W2c[0:128, :], W2c[128:256, :]], axis=1).astype(bf)
    b1c = np.ascontiguousarray(
        np.asarray(b_attn1, np.float32).reshape(2, 128).T
    )  # (128, 2)
    blobB = np.zeros((128, 512), dtype=bf)
    blobB[:, 0:64] = Wp2I
    blobB[:, 64:320] = W1dup
    blobB[:, 320:448] = W2cat

    blobF = np.zeros((128, 1348), dtype=np.float32)
    blobF[0:DIM, 0:1024] = xT
    blobF[0:DIM, 1152:1216] = Wq
    blobF[0:DIM, 1216:1280] = Wk
    blobF[0:DIM, 1280:1344] = Wv
    blobF[0:DIM, 1344:1345] = np.asarray(b_pos1, np.float32).reshape(DIM, 1)
    blobF[0:DIM, 1345:1346] = np.asarray(b_pos2, np.float32).reshape(DIM, 1)
    blobF[:, 1346:1348] = b1c

    in_maps = []
    for c in range(NCORES):
        bFc = blobF.copy()
        bFc[0:DIM, 1024:1152] = xT[:, c * ROWS:(c + 1) * ROWS]
        bPc = np.zeros((3, 1216), np.float32)
        bPc[:, 0:1024] = pT
        bPc[:, 1024:1152] = pT[:, c * ROWS:(c + 1) * ROWS]
        bPc[:, 1152:1216] = np.asarray(W_pos1, np.float32)
        in_maps.append({"blobP": bPc, "blobF": bFc, "blobB": blobB})
    return in_maps


def kernel(x, pos, W_qkv, W_pos1, b_pos1, W_pos2, b_pos2,
           W_attn1, b_attn1, W_attn2, b_attn2, _want_trace=False):
    if "nc" not in _CACHE:
        _CACHE["nc"] = _build_program()
    nc = _CACHE["nc"]
    in_maps = _prep_inputs(x, pos, W_qkv, W_pos1, b_pos1, W_pos2, b_pos2,
                           W_attn1, b_attn1, W_attn2, b_attn2)
    res = run_bass_kernel_spmd(
        nc, in_maps, core_ids=list(range(NCORES)), trace=_want_trace
    )
    _CACHE["last_results"] = res
    out = np.empty((N, DIM), np.float32)
    for c in range(NCORES):
        agg = np.asarray(res.results[c]["agg_out"], np.float32)  # (128, 64)
        out[c * ROWS:c * ROWS + NPAIR, :] = agg[0:DIM, :].T
        out[c * ROWS + NPAIR:(c + 1) * ROWS, :] = agg[DIM:128, :].T
    return out.reshape(1, N, DIM)


# revision 56
# speedup vs baseline: 1.1108x; 1.0224x over previous
"""Trainium2 Bass kernel for nn_PointTransformerLayer (N=1024, dim=64, 8 cores).

Sharding: query rows i are split across 8 cores (128 rows each, flash-attention
style); k/v/pos and all weights are replicated to every core host-side.

Math (per core, rows i in its slice, all j):
  a_i  = p_i @ W_pos1 + b_pos1            (per-i, precomputed)
  bn_j = -(p_j @ W_pos1)                  (per-j, precomputed)
  R    = relu(a_i + bn_j)                 -> bf16                    [pos MLP l1]
  U    = R @ W_pos2 - k_j + q_i + b_pos2  (k folded into the matmul via -I rows,
                                           q_i + b_pos2 added as evac bias)
  H    = relu(U @ W_attn1 + b_attn1)      -> bf16
  sim  = H @ W_attn2                      (b_attn2 dropped: softmax-invariant)
  E    = exp(sim)  (no max-sub: |sim| < ~1)
  agg  = (sum_j E*U + sum_j E*(v_j+k_j)) / sum_j E - q_i
       == softmax(sim) . (v_j + rpe)  since v_e = U + (v+k) - q

Layouts are feature-major: [features on partitions, points on free dim].
Two query rows are processed per iteration by packing their 64-wide feature
tensors into the 128 partitions (group A = rows 0..63, group B = rows 64..127
of the core's slice).

Optimizations vs the first working version:
  - den (sum_j E) comes free from the exp ACTIVATE's accum_out, so the DVE
    fold chain only processes the E*V product (P-only folds + FD-256 reduce).
  - U evac is a single FD-1024 ACTIVATE (2-bank PSUM read); sim/exp likewise
    merged (split_sim=False) to pay the accumulator-read cost once.
  - The elementwise tail (V add, E*V, folds, reduce) runs once per TWO
    iterations at FD=2048 to amortize per-op overhead (batch2).
  - All inputs arrive as two packed blobs (2 landing DMAs instead of 14).
  - H evacs are split ACT/DVE per h_act_pattern to balance the two engines.
(GpSimd offload and the fused scalar_tensor_tensor/tensor_tensor_reduce ops
were measured/found unusable on this runtime: 15us/op, 5.7us/op, and a
runtime failure respectively.)
"""

import sys

sys.path.insert(0, "/opt/trn_rl_repo")

import numpy as np
import ml_dtypes

import concourse.bass as bass
import concourse.bacc as bacc
import concourse.mybir as mybir
import concourse.tile as tile
from concourse.bass_utils import run_bass_kernel_spmd
from concourse.tile_rust import add_dep_helper

F32 = mybir.dt.float32
BF16 = mybir.dt.bfloat16
AF = mybir.ActivationFunctionType
OP = mybir.AluOpType

N = 1024
DIM = 64
HID = 256  # DIM * ATTN_MULT
NCORES = 8
ROWS = N // NCORES  # 128 query rows per core
NPAIR = ROWS // 2  # 64 iterations, 2 rows (A/B groups) each

_CACHE = {}
_CONFIG = {
    "r_pool": False,    # R tensor_scalar on GpSimd (measured 15us/op — keep off)
    "h_act_pattern": (2, 2, 2, 3),  # per-iteration count of H evacs on ACT
    "h_act": 2,         # fallback when h_act_pattern unset
    "split_sim": True,  # per-jc sim PSUM tiles + exp (better overlap)
    "batch2": True,     # run the elementwise tail once per TWO iterations
}


def _setup_phase(nc, tc, t):
    """Small precompute matmuls. Ordered so the tensors gating iteration 0
    (bn2/aA -> R, then Rbuf kT halves -> U) materialize first."""
    with tc.tile_pool(name="setup_ps", bufs=2, space="PSUM") as spool:
        # bn = -(p@Wpos1) over all j (bf16 both halves)   [gates R(0)]
        for jc in range(2):
            ps = spool.tile([DIM, 512], F32, tag="bt_ps", name="ps")
            nc.tensor.matmul(ps[:], t.Wpos1[:], t.pT[:, jc * 512:(jc + 1) * 512])
            nc.scalar.activation(
                t.bn2[0:DIM, jc * 512:(jc + 1) * 512], ps[:], AF.Identity,
                bias=0.0, scale=-1.0,
            )
        nc.vector.tensor_copy(t.bn2[DIM:128, :], t.bn2[0:DIM, :])
        # a-cols for this core's rows: a = p_i@Wpos1 + b_pos1  [gates R(0)]
        # stacked: partitions 0:64 = rows 0..63 (group A), 64:128 = group B
        aps = spool.tile([DIM, ROWS], F32, tag="a_ps", name="aps")
        nc.tensor.matmul(aps[:], t.Wpos1[:], t.pTs[:])
        nc.scalar.activation(t.aA[:], aps[:, 0:NPAIR], AF.Identity, bias=t.bpos1[:])
        nc.scalar.activation(t.aB[:], aps[:, NPAIR:ROWS], AF.Identity, bias=t.bpos1[:])
        # k^T, v^T over all j  (f32 matmuls)
        for jc in range(2):
            ps = spool.tile([DIM, 512], F32, tag="kv_ps", name="ps")
            nc.tensor.matmul(ps[:], t.Wk[:], t.xT[:, jc * 512:(jc + 1) * 512])
            nc.scalar.copy(t.tmp_kT[:, jc * 512:(jc + 1) * 512], ps[:])
            ps2 = spool.tile([DIM, 512], F32, tag="kv_ps", name="ps2")
            nc.tensor.matmul(ps2[:], t.Wv[:], t.xT[:, jc * 512:(jc + 1) * 512])
            nc.scalar.copy(t.tmp_vT[:, jc * 512:(jc + 1) * 512], ps2[:])
        # static k^T (bf16) into partitions 64:128 of the R buffers [gates U(0)]
        for RA, RB in t.Rbufs:
            nc.vector.tensor_copy(RA[DIM:128, :], t.tmp_kT[:])
            nc.vector.tensor_copy(RB[DIM:128, :], t.tmp_kT[:])
        # vk = v + k (bf16, both partition halves)
        nc.vector.tensor_tensor(
            out=t.vk2[0:DIM, :], in0=t.tmp_kT[:], in1=t.tmp_vT[:], op=OP.add
        )
        nc.vector.tensor_copy(t.vk2[DIM:128, :], t.vk2[0:DIM, :])
        if _CONFIG.get("batch2"):
            nc.vector.tensor_copy(t.vk4[:, 0:N], t.vk2[:])
            nc.vector.tensor_copy(t.vk4[:, N:2 * N], t.vk2[:])
        # q-cols for this core's rows
        qps = spool.tile([DIM, ROWS], F32, tag="q_ps", name="qps")
        nc.tensor.matmul(qps[:], t.Wq[:], t.xTs[:])
        nc.scalar.copy(t.qT2[0:DIM, :], qps[:, 0:NPAIR])
        nc.scalar.copy(t.qT2[DIM:128, :], qps[:, NPAIR:ROWS])
        nc.scalar.activation(t.qb2[0:DIM, :], qps[:, 0:NPAIR], AF.Identity, bias=t.bpos2[:])
        nc.scalar.activation(t.qb2[DIM:128, :], qps[:, NPAIR:ROWS], AF.Identity, bias=t.bpos2[:])


def _emit_R(nc, t, m):
    """R(m) = relu(a_m + bn_j) -> bf16 into the rotating R buffers (kT lives
    statically in partitions 64:128). Emitted `r_ahead` iterations early so
    the U matmul never waits on it."""
    RA, RB = t.Rbufs[m % len(t.Rbufs)]
    nc.vector.tensor_scalar(
        out=RA[0:DIM, :], in0=t.bn2[0:DIM, :],
        scalar1=t.aA[:, m:m + 1], scalar2=0.0, op0=OP.add, op1=OP.max,
    )
    nc.vector.tensor_scalar(
        out=RB[0:DIM, :], in0=t.bn2[0:DIM, :],
        scalar1=t.aB[:, m:m + 1], scalar2=0.0, op0=OP.add, op1=OP.max,
    )


def _pair_iteration(nc, t, pools, m):
    """One iteration: two query rows (groups A/B) against all 1024 j."""
    wpool, hpool, upool, hpspool, simpool = pools
    RA, RB = t.Rbufs[m % len(t.Rbufs)]
    # U = [Wpos2; -I]^T @ [R; kT]  (single-mm groups, col-tiled concurrent)
    half = m % 2 if _CONFIG.get("batch2") else 0
    if _CONFIG.get("batch2"):
        if half == 0:
            t.cur_U2 = wpool.tile([128, 2 * N], BF16, tag="U2", name="U2")
            t.cur_EP2 = wpool.tile([128, 2 * N], BF16, tag="EP2", name="EP2")
        U_sb = t.cur_U2[:, half * N:(half + 1) * N]
    else:
        U_sb = wpool.tile([128, N], BF16, tag="U_sb", name="U_sb")
    U_ps = upool.tile([128, N], F32, tag="U_ps", name="U_ps", bufs=1)
    for jc in range(2):
        for g, Rt in ((0, RA), (1, RB)):
            nc.tensor.matmul(
                U_ps[g * DIM:(g + 1) * DIM, jc * 512:(jc + 1) * 512],
                t.Wp2I[:],
                Rt[:, jc * 512:(jc + 1) * 512],
                tile_position=(0, g * DIM),
            )
    nc.scalar.activation(
        U_sb[:], U_ps[:], AF.Identity, bias=t.qb2[:, m:m + 1]
    )
    # H = relu(U @ W_attn1 + b_attn1) -> bf16
    # one [128,1024] PSUM tile per (hb, jc): groups A/B in free-dim halves,
    # so the whole tile shares one per-partition bias (b_attn1[hb]) and the
    # evac is a single FD-1024 op, split between ACT and DVE per h_act.
    pat = _CONFIG.get("h_act_pattern")
    if pat:
        n_act = pat[m % len(pat)]
    else:
        h_act = _CONFIG.get("h_act", 2)
        n_act = int(h_act) + (1 if (h_act % 1) and (m % 2 == 1) else 0)
    H_sbs = {}
    evac_i = 0
    for hb in range(2):
        for jc in range(2):
            H_ps = hpspool.tile([128, 2 * 512], F32, tag="H_ps", name="H_ps")
            for g in range(2):
                nc.tensor.matmul(
                    H_ps[:, g * 512:(g + 1) * 512],
                    t.W1[g * DIM:(g + 1) * DIM, hb * 128:(hb + 1) * 128],
                    U_sb[g * DIM:(g + 1) * DIM, jc * 512:(jc + 1) * 512],
                    tile_position=(g * DIM, 0),
                )
            H_sb = hpool.tile([128, 2 * 512], BF16, tag="H_sb", name="H_sb")
            # interleave ACT/DVE evacs (ACT also has U/exp work)
            act_sets = {0: (), 1: (1,), 2: (0, 2), 3: (0, 2, 3), 4: (0, 1, 2, 3)}
            use_act = evac_i in act_sets[n_act]
            if use_act:
                nc.scalar.activation(
                    H_sb[:], H_ps[:], AF.Relu, bias=t.b1[:, hb:hb + 1]
                )
            else:
                nc.vector.tensor_scalar(
                    out=H_sb[:], in0=H_ps[:],
                    scalar1=t.b1[:, hb:hb + 1], scalar2=0.0,
                    op0=OP.add, op1=OP.max,
                )
            evac_i += 1
            H_sbs[(hb, jc)] = H_sb
    # sim = H @ W_attn2  (2-mm accumulation chains; keep each PSUM bank's
    # chains strictly sequential: group A completes before group B starts).
    # One 1-bank tile per jc half + per-jc exp (accum_out -> den) lets PE
    # run ahead of ACT.
    if _CONFIG.get("batch2"):
        EP = t.cur_EP2[:, half * N:(half + 1) * N]
    else:
        EP = wpool.tile([128, N], BF16, tag="EP", name="EP")
    if _CONFIG.get("split_sim", True):
        sim_tiles = [
            simpool.tile([128, 512], F32, tag="SIM_ps", name="SIM_ps")
            for _ in range(2)
        ]
    else:
        big = simpool.tile([128, N], F32, tag="SIM_ps", name="SIM_ps", bufs=1)
        sim_tiles = [big[:, 0:512], big[:, 512:1024]]
    for jc in range(2):
        SIM_ps = sim_tiles[jc]
        prev_last = None
        for g in range(2):
            insts = []
            for hb in range(2):
                inst = nc.tensor.matmul(
                    SIM_ps[g * DIM:(g + 1) * DIM, :],
                    t.W2[:, hb * DIM:(hb + 1) * DIM],
                    H_sbs[(hb, jc)][:, g * 512:(g + 1) * 512],
                    start=(hb == 0),
                    stop=(hb == 1),
                    tile_position=(0, g * DIM),
                )
                insts.append(inst)
            if prev_last is not None:
                add_dep_helper(
                    insts[0].ins, prev_last.ins, False,
                    "psum zero-region chain order",
                )
            prev_last = insts[1]
        if _CONFIG.get("split_sim", True):
            nc.scalar.activation(
                EP[:, jc * 512:(jc + 1) * 512], SIM_ps[:], AF.Exp,
                accum_out=t.dens[jc][:, m:m + 1],
            )
    if not _CONFIG.get("split_sim", True):
        nc.scalar.activation(
            EP[:, 0:N], big[:], AF.Exp, accum_out=t.dens[0][:, m:m + 1]
        )
    # v_e (mod q) = U + (v+k); P = E * V; then fold P pairwise at 2x and do
    # the final 1x tensor_reduce on only 256 elements per row. den comes from
    # the exp's accum_out, so the folds only process the product. With batch2
    # the whole tail runs once per TWO iterations at FD=2048.
    if _CONFIG.get("batch2"):
        if half == 1:
            V2 = wpool.tile([128, 2 * N], BF16, tag="V2", name="V2")
            nc.vector.tensor_tensor(
                out=V2[:], in0=t.cur_U2[:], in1=t.vk4[:], op=OP.add
            )
            SCR2 = wpool.tile([128, 2 * N], BF16, tag="SCR2", name="SCR2")
            nc.vector.tensor_tensor(
                out=SCR2[:], in0=t.cur_EP2[:], in1=V2[:], op=OP.mult
            )
            s3 = SCR2.rearrange("p (k n) -> p k n", k=2)
            F1 = wpool.tile([128, N], BF16, tag="F1", name="F1")
            f1 = F1.rearrange("p (k n) -> p k n", k=2)
            nc.vector.tensor_tensor(
                out=f1[:, :, :], in0=s3[:, :, 0:512], in1=s3[:, :, 512:1024],
                op=OP.add,
            )
            F2 = wpool.tile([128, N // 2], BF16, tag="F2", name="F2")
            f2 = F2.rearrange("p (k n) -> p k n", k=2)
            nc.vector.tensor_tensor(
                out=f2[:, :, :], in0=f1[:, :, 0:256], in1=f1[:, :, 256:512],
                op=OP.add,
            )
            F3 = wpool.tile([128, N // 4], BF16, tag="F3", name="F3")
            f3 = F3.rearrange("p (k n) -> p k n", k=2)
            nc.vector.tensor_tensor(
                out=f3[:, :, :], in0=f2[:, :, 0:128], in1=f2[:, :, 128:256],
                op=OP.add,
            )
            nc.vector.tensor_reduce(
                out=t.numU[:, m - 1:m + 1], in_=f3[:, :, :],
                axis=mybir.AxisListType.X, op=OP.add,
            )
    else:
        V_sb = wpool.tile([128, N], BF16, tag="V_sb", name="V_sb")
        nc.vector.tensor_tensor(out=V_sb[:], in0=U_sb[:], in1=t.vk2[:], op=OP.add)
        SCR1 = wpool.tile([128, N], BF16, tag="SCR1", name="SCR1")
        nc.vector.tensor_tensor(
            out=SCR1[:], in0=EP[:], in1=V_sb[:], op=OP.mult
        )
        F1 = wpool.tile([128, N // 2], BF16, tag="F1", name="F1")
        nc.vector.tensor_tensor(
            out=F1[:], in0=SCR1[:, 0:512], in1=SCR1[:, 512:1024], op=OP.add
        )
        F2 = wpool.tile([128, N // 4], BF16, tag="F2", name="F2")
        nc.vector.tensor_tensor(
            out=F2[:], in0=F1[:, 0:256], in1=F1[:, 256:512], op=OP.add
        )
        nc.vector.tensor_reduce(
            out=t.numU[:, m:m + 1], in_=F2[:],
            axis=mybir.AxisListType.X, op=OP.add,
        )


class _Tiles:
    pass


def _build_program(repeat=1):
    """Build the Bass program (same program for all 8 cores; per-core data
    comes from in_maps). Returns the Bass object. `repeat` re-runs the main
    loop N times inside the NEFF (for slope-based device timing)."""
    nc = bacc.Bacc("TRN2", debug=False, num_devices=1, target_bir_lowering=False)

    # ---- DRAM I/O ----
    # All inputs are packed host-side into two blobs (one per dtype) so the
    # kernel head issues 2 landing DMAs instead of 14.
    d_bP = nc.dram_tensor("blobP", [3, 1216], F32, kind="ExternalInput")
    d_bF = nc.dram_tensor("blobF", [128, 1348], F32, kind="ExternalInput")
    d_bB = nc.dram_tensor("blobB", [128, 512], BF16, kind="ExternalInput")
    d_out = nc.dram_tensor("agg_out", [128, NPAIR], F32, kind="ExternalOutput")

    with tile.TileContext(nc) as tc:
        with (
            tc.tile_pool(name="const", bufs=1) as cpool,
            tc.tile_pool(name="work", bufs=6) as wpool,
            tc.tile_pool(name="hsb", bufs=10) as hpool,
        ):
            t = _Tiles()
            # ---------------- persistent SBUF ----------------
            for name, shape, dt in (
                ("blobP", [3, 1216], F32),
                ("blobF", [128, 1348], F32), ("blobB", [128, 512], BF16),
                ("vk2", [128, N], BF16), ("vk4", [128, 2 * N], BF16),
                ("bn2", [128, N], BF16),
                ("aA", [DIM, NPAIR], F32), ("aB", [DIM, NPAIR], F32),
                ("qT2", [128, NPAIR], F32), ("qb2", [128, NPAIR], F32),
                ("den0", [128, NPAIR], F32), ("den1", [128, NPAIR], F32),
                ("numU", [128, NPAIR], F32),
                ("tmp_kT", [DIM, N], F32), ("tmp_vT", [DIM, N], F32),
                ("warm", [128, 8], F32),
                ("recS0", [128, NPAIR], F32), ("agg", [128, NPAIR], F32),
            ):
                setattr(t, name, cpool.tile(shape, dt, tag=name, name=name))
            # blob slice views (same layout as _prep_inputs)
            bF, bB = t.blobF, t.blobB
            t.xT = bF[0:DIM, 0:1024]
            t.xTs = bF[0:DIM, 1024:1152]
            t.Wq = bF[0:DIM, 1152:1216]
            t.Wk = bF[0:DIM, 1216:1280]
            t.Wv = bF[0:DIM, 1280:1344]
            t.bpos1 = bF[0:DIM, 1344:1345]
            t.bpos2 = bF[0:DIM, 1345:1346]
            t.b1 = bF[:, 1346:1348]
            t.pT = t.blobP[:, 0:1024]
            t.pTs = t.blobP[:, 1024:1152]
            t.Wpos1 = t.blobP[:, 1152:1216]
            t.Wp2I = bB[:, 0:64]
            t.W1 = bB[:, 64:320]
            t.W2 = bB[:, 320:448]
            t.dens = [t.den0, t.den1]
            t.Rbufs = [
                (cpool.tile([128, N], BF16, tag=f"RA{p}", name=f"RA{p}"),
                 cpool.tile([128, N], BF16, tag=f"RB{p}", name=f"RB{p}"))
                for p in range(3)
            ]

            # ---------------- DMA loads ----------------
            # the small pos blob lands first: it gates the bn/a matmuls that
            # feed R(0) and hence the whole pipeline
            nc.sync.dma_start(t.blobP[:], d_bP.ap())
            nc.sync.dma_start(t.blobF[:], d_bF.ap())
            nc.sync.dma_start(t.blobB[:], d_bB.ap())

            # preload the exp table set early (one-time ~2.7us)
            nc.gpsimd.memset(t.warm[:], 0.0)
            nc.scalar.activation(t.warm[:], t.warm[:], AF.Exp)

            _setup_phase(nc, tc, t)
            if not _CONFIG.get("split_sim", True):
                nc.vector.memset(t.den1[:], 0.0)

            # ---------------- main loop over row pairs ----------------
            with (
                tc.tile_pool(name="u_ps", bufs=2, space="PSUM") as upool,
                tc.tile_pool(name="h_ps", bufs=2, space="PSUM") as hpspool,
                tc.tile_pool(name="s_ps", bufs=2, space="PSUM") as simpool,
            ):
                pools = (wpool, hpool, upool, hpspool, simpool)
                r_ahead = _CONFIG.get("r_ahead", 2)
                for _r in range(repeat):
                    for m in range(min(r_ahead, NPAIR)):
                        _emit_R(nc, t, m)
                    for m in range(NPAIR):
                        if m + r_ahead < NPAIR:
                            _emit_R(nc, t, m + r_ahead)
                        _pair_iteration(nc, t, pools, m)

            # ---------------- finalize ----------------
            nc.vector.tensor_tensor(
                out=t.den0[:], in0=t.den0[:], in1=t.den1[:], op=OP.add
            )
            nc.vector.reciprocal(t.recS0[:], t.den0[:])
            nc.vector.tensor_tensor(
                out=t.agg[:], in0=t.numU[:], in1=t.recS0[:], op=OP.mult
            )
            nc.vector.tensor_tensor(
                out=t.agg[:], in0=t.agg[:], in1=t.qT2[:], op=OP.subtract
            )
            nc.sync.dma_start(d_out.ap(), t.agg[:])

    nc.compile()
    return nc


def _prep_inputs(x, pos, W_qkv, W_pos1, b_pos1, W_pos2, b_pos2,
                 W_attn1, b_attn1, W_attn2, b_attn2):
    """Host-side data prep: slicing/transposes/weight packing (no O(N^2) math)."""
    bf = ml_dtypes.bfloat16
    x2 = np.ascontiguousarray(np.asarray(x, np.float32).reshape(N, DIM))
    p2 = np.ascontiguousarray(np.asarray(pos, np.float32).reshape(N, 3))
    xT = np.ascontiguousarray(x2.T)  # (64, N)
    pT = np.ascontiguousarray(p2.T)  # (3, N)
    W_qkv = np.asarray(W_qkv, np.float32)
    Wq = np.ascontiguousarray(W_qkv[:, 0:DIM])
    Wk = np.ascontiguousarray(W_qkv[:, DIM:2 * DIM])
    Wv = np.ascontiguousarray(W_qkv[:, 2 * DIM:3 * DIM])
    Wp2I = np.concatenate(
        [np.asarray(W_pos2, np.float32), -np.eye(DIM, dtype=np.float32)], axis=0
    ).astype(bf)  # (128, 64): [Wpos2; -I] for the combined U matmul
    W1dup = np.concatenate(
        [np.asarray(W_attn1, np.float32)] * 2, axis=0
    ).astype(bf)  # (128, 256)
    W2c = np.asarray(W_attn2, np.float32)
    W2cat = np.concatenate([W2c[0:128, :], W2c[128:256, :]], axis=1).astype(bf)
    b1c = np.ascontiguousarray(
        np.asarray(b_attn1, np.float32).reshape(2, 128).T
    )  # (128, 2)
    blobB = np.zeros((128, 512), dtype=bf)
    blobB[:, 0:64] = Wp2I
    blobB[:, 64:320] = W1dup
    blobB[:, 320:448] = W2cat

    blobF = np.zeros((128, 1348), dtype=np.float32)
    blobF[0:DIM, 0:1024] = xT
    blobF[0:DIM, 1152:1216] = Wq
    blobF[0:DIM, 1216:1280] = Wk
    blobF[0:DIM, 1280:1344] = Wv
    blobF[0:DIM, 1344:1345] = np.asarray(b_pos1, np.float32).reshape(DIM, 1)
    blobF[0:DIM, 1345:1346] = np.asarray(b_pos2, np.float32).reshape(DIM, 1)
    blobF[:, 1346:1348] = b1c

    in_maps = []
    for c in range(NCORES):
        bFc = blobF.copy()
        bFc[0:DIM, 1024:1152] = xT[:, c * ROWS:(c + 1) * ROWS]
        bPc = np.zeros((3, 1216), np.float32)
        bPc[:, 0:1024] = pT
        bPc[:, 1024:1152] = pT[:, c * ROWS:(c + 1) * ROWS]
        bPc[:, 1152:1216] = np.asarray(W_pos1, np.float32)
        in_maps.append({"blobP": bPc, "blobF": bFc, "blobB": blobB})
    return in_maps


def kernel(x, pos, W_qkv, W_pos1, b_pos1, W_pos2, b_pos2,
           W_attn1, b_attn1, W_attn2, b_attn2, _want_trace=False):
    if "nc" not in _CACHE:
        _CACHE["nc"] = _build_program()
    nc = _CACHE["nc"]
    in_maps = _prep_inputs(x, pos, W_qkv, W_pos1, b_pos1, W_pos2, b_pos2,
                           W_attn1, b_attn1, W_attn2, b_attn2)
    res = run_bass_kernel_spmd(
        nc, in_maps, core_ids=list(range(NCORES)), trace=_want_trace
    )
    _CACHE["last_results"] = res
    out = np.empty((N, DIM), np.float32)
    for c in range(NCORES):
        agg = np.asarray(res.results[c]["agg_out"], np.float32)  # (128, 64)
        out[c * ROWS:c * ROWS + NPAIR, :] = agg[0:DIM, :].T
        out[c * ROWS + NPAIR:(c + 1) * ROWS, :] = agg[DIM:128, :].T
    return out.reshape(1, N, DIM)


# revision 61
# speedup vs baseline: 1.1237x; 1.0116x over previous
"""Trainium2 Bass kernel for nn_PointTransformerLayer (N=1024, dim=64, 8 cores).

Sharding: query rows i are split across 8 cores (128 rows each, flash-attention
style); k/v/pos and all weights are replicated to every core host-side.

Math (per core, rows i in its slice, all j):
  a_i  = p_i @ W_pos1 + b_pos1            (per-i, precomputed)
  bn_j = -(p_j @ W_pos1)                  (per-j, precomputed)
  R    = relu(a_i + bn_j)                 -> bf16                    [pos MLP l1]
  U    = R @ W_pos2 - k_j + q_i + b_pos2  (k folded into the matmul via -I rows,
                                           q_i + b_pos2 added as evac bias)
  H    = relu(U @ W_attn1 + b_attn1)      -> bf16
  sim  = H @ W_attn2                      (b_attn2 dropped: softmax-invariant)
  E    = exp(sim)  (no max-sub: |sim| < ~1)
  agg  = (sum_j E*U + sum_j E*(v_j+k_j)) / sum_j E - q_i
       == softmax(sim) . (v_j + rpe)  since v_e = U + (v+k) - q

Layouts are feature-major: [features on partitions, points on free dim].
Two query rows are processed per iteration by packing their 64-wide feature
tensors into the 128 partitions (group A = rows 0..63, group B = rows 64..127
of the core's slice).

Optimizations vs the first working version:
  - den (sum_j E) comes free from the exp ACTIVATE's accum_out, so the DVE
    fold chain only processes the E*V product (P-only folds + FD-256 reduce).
  - U evac is a single FD-1024 ACTIVATE (2-bank PSUM read); sim/exp likewise
    merged (split_sim=False) to pay the accumulator-read cost once.
  - The elementwise tail (V add, E*V, folds, reduce) runs once per TWO
    iterations at FD=2048 to amortize per-op overhead (batch2).
  - All inputs arrive as two packed blobs (2 landing DMAs instead of 14).
  - H evacs are split ACT/DVE per h_act_pattern to balance the two engines.
(GpSimd offload and the fused scalar_tensor_tensor/tensor_tensor_reduce ops
were measured/found unusable on this runtime: 15us/op, 5.7us/op, and a
runtime failure respectively.)
"""

import sys

sys.path.insert(0, "/opt/trn_rl_repo")

import numpy as np
import ml_dtypes

import concourse.bass as bass
import concourse.bacc as bacc
import concourse.mybir as mybir
import concourse.tile as tile
from concourse.bass_utils import run_bass_kernel_spmd
from concourse.tile_rust import add_dep_helper

F32 = mybir.dt.float32
BF16 = mybir.dt.bfloat16
AF = mybir.ActivationFunctionType
OP = mybir.AluOpType

N = 1024
DIM = 64
HID = 256  # DIM * ATTN_MULT
NCORES = 8
ROWS = N // NCORES  # 128 query rows per core
NPAIR = ROWS // 2  # 64 iterations, 2 rows (A/B groups) each

_CACHE = {}
_CONFIG = {
    "r_pool": False,    # R tensor_scalar on GpSimd (measured 15us/op — keep off)
    "h_act_pattern": (2, 2, 2, 3),  # per-iteration count of H evacs on ACT
    "h_act": 2,         # fallback when h_act_pattern unset
    "split_sim": True,  # per-jc sim PSUM tiles + exp (better overlap)
    "batch2": True,     # run the elementwise tail once per TWO iterations
}


def _setup_phase(nc, tc, t):
    """Small precompute matmuls. Ordered so the tensors gating iteration 0
    (bn2/aA -> R, then Rbuf kT halves -> U) materialize first."""
    with tc.tile_pool(name="setup_ps", bufs=2, space="PSUM") as spool:
        # bn = -(p@Wpos1) over all j (bf16 both halves)   [gates R(0)]
        for jc in range(2):
            ps = spool.tile([DIM, 512], F32, tag="bt_ps", name="ps")
            nc.tensor.matmul(ps[:], t.Wpos1[:], t.pT[:, jc * 512:(jc + 1) * 512])
            nc.scalar.activation(
                t.bn2[0:DIM, jc * 512:(jc + 1) * 512], ps[:], AF.Identity,
                bias=0.0, scale=-1.0,
            )
        # (bn2 partitions 64:128 are never read — R uses [0:64] only)
        # a-cols for this core's rows: a = p_i@Wpos1 + b_pos1  [gates R(0)]
        # stacked: partitions 0:64 = rows 0..63 (group A), 64:128 = group B
        aps = spool.tile([DIM, ROWS], F32, tag="a_ps", name="aps")
        nc.tensor.matmul(aps[:], t.Wpos1[:], t.pTs[:])
        nc.scalar.activation(t.aA[:], aps[:, 0:NPAIR], AF.Identity, bias=t.bpos1[:])
        nc.scalar.activation(t.aB[:], aps[:, NPAIR:ROWS], AF.Identity, bias=t.bpos1[:])
        # k^T, v^T over all j. k evacs straight to bf16 into Rbuf[0]'s static
        # half; the other R buffers are filled by cheap 4x-mode bf16 copies
        # (the old fp32->bf16 fan-out ran at 1x and cost ~7us of head time).
        RA0, RB0 = t.Rbufs[0]
        for jc in range(2):
            ps = spool.tile([DIM, 512], F32, tag="kv_ps", name="ps")
            nc.tensor.matmul(ps[:], t.Wk[:], t.xT[:, jc * 512:(jc + 1) * 512])
            nc.scalar.copy(RA0[DIM:128, jc * 512:(jc + 1) * 512], ps[:])
            ps2 = spool.tile([DIM, 512], F32, tag="kv_ps", name="ps2")
            nc.tensor.matmul(ps2[:], t.Wv[:], t.xT[:, jc * 512:(jc + 1) * 512])
            nc.scalar.copy(t.tmp_vT[:, jc * 512:(jc + 1) * 512], ps2[:])
        nc.vector.tensor_copy(RB0[DIM:128, :], RA0[DIM:128, :])
        for RA, RB in t.Rbufs[1:]:
            nc.vector.tensor_copy(RA[DIM:128, :], RA0[DIM:128, :])
            nc.vector.tensor_copy(RB[DIM:128, :], RA0[DIM:128, :])
        # vk = v + k (bf16, both partition halves); tensor_tensor needs both
        # inputs at the same base partition, so stage k at base 0 first
        nc.vector.tensor_copy(t.kB[:], RA0[DIM:128, :])
        nc.vector.tensor_tensor(
            out=t.vk2[0:DIM, :], in0=t.kB[:], in1=t.tmp_vT[:], op=OP.add
        )
        nc.vector.tensor_copy(t.vk2[DIM:128, :], t.vk2[0:DIM, :])
        if _CONFIG.get("batch2"):
            nc.vector.tensor_copy(t.vk4[:, 0:N], t.vk2[:])
            nc.vector.tensor_copy(t.vk4[:, N:2 * N], t.vk2[:])
        # q-cols for this core's rows
        qps = spool.tile([DIM, ROWS], F32, tag="q_ps", name="qps")
        nc.tensor.matmul(qps[:], t.Wq[:], t.xTs[:])
        nc.scalar.copy(t.qT2[0:DIM, :], qps[:, 0:NPAIR])
        nc.scalar.copy(t.qT2[DIM:128, :], qps[:, NPAIR:ROWS])
        nc.scalar.activation(t.qb2[0:DIM, :], qps[:, 0:NPAIR], AF.Identity, bias=t.bpos2[:])
        nc.scalar.activation(t.qb2[DIM:128, :], qps[:, NPAIR:ROWS], AF.Identity, bias=t.bpos2[:])


def _emit_R(nc, t, m):
    """R(m) = relu(a_m + bn_j) -> bf16 into the rotating R buffers (kT lives
    statically in partitions 64:128). Emitted `r_ahead` iterations early so
    the U matmul never waits on it."""
    RA, RB = t.Rbufs[m % len(t.Rbufs)]
    nc.vector.tensor_scalar(
        out=RA[0:DIM, :], in0=t.bn2[0:DIM, :],
        scalar1=t.aA[:, m:m + 1], scalar2=0.0, op0=OP.add, op1=OP.max,
    )
    nc.vector.tensor_scalar(
        out=RB[0:DIM, :], in0=t.bn2[0:DIM, :],
        scalar1=t.aB[:, m:m + 1], scalar2=0.0, op0=OP.add, op1=OP.max,
    )


def _pair_iteration(nc, t, pools, m):
    """One iteration: two query rows (groups A/B) against all 1024 j."""
    wpool, hpool, upool, hpspool, simpool = pools
    RA, RB = t.Rbufs[m % len(t.Rbufs)]
    # U = [Wpos2; -I]^T @ [R; kT]  (single-mm groups, col-tiled concurrent)
    half = m % 2 if _CONFIG.get("batch2") else 0
    if _CONFIG.get("batch2"):
        if half == 0:
            t.cur_U2 = wpool.tile([128, 2 * N], BF16, tag="U2", name="U2")
            t.cur_EP2 = wpool.tile([128, 2 * N], BF16, tag="EP2", name="EP2")
        U_sb = t.cur_U2[:, half * N:(half + 1) * N]
    else:
        U_sb = wpool.tile([128, N], BF16, tag="U_sb", name="U_sb")
    U_ps = upool.tile([128, N], F32, tag="U_ps", name="U_ps", bufs=1)
    for jc in range(2):
        for g, Rt in ((0, RA), (1, RB)):
            nc.tensor.matmul(
                U_ps[g * DIM:(g + 1) * DIM, jc * 512:(jc + 1) * 512],
                t.Wp2I[:],
                Rt[:, jc * 512:(jc + 1) * 512],
                tile_position=(0, g * DIM),
            )
    nc.scalar.activation(
        U_sb[:], U_ps[:], AF.Identity, bias=t.qb2[:, m:m + 1]
    )
    # H = relu(U @ W_attn1 + b_attn1) -> bf16
    # one [128,1024] PSUM tile per (hb, jc): groups A/B in free-dim halves,
    # so the whole tile shares one per-partition bias (b_attn1[hb]) and the
    # evac is a single FD-1024 op, split between ACT and DVE per h_act.
    pat = _CONFIG.get("h_act_pattern")
    if pat:
        n_act = pat[m % len(pat)]
    else:
        h_act = _CONFIG.get("h_act", 2)
        n_act = int(h_act) + (1 if (h_act % 1) and (m % 2 == 1) else 0)
    H_sbs = {}
    evac_i = 0
    for hb in range(2):
        for jc in range(2):
            H_ps = hpspool.tile([128, 2 * 512], F32, tag="H_ps", name="H_ps")
            for g in range(2):
                nc.tensor.matmul(
                    H_ps[:, g * 512:(g + 1) * 512],
                    t.W1[g * DIM:(g + 1) * DIM, hb * 128:(hb + 1) * 128],
                    U_sb[g * DIM:(g + 1) * DIM, jc * 512:(jc + 1) * 512],
                    tile_position=(g * DIM, 0),
                )
            H_sb = hpool.tile([128, 2 * 512], BF16, tag="H_sb", name="H_sb")
            # interleave ACT/DVE evacs (ACT also has U/exp work)
            act_sets = {0: (), 1: (1,), 2: (0, 2), 3: (0, 2, 3), 4: (0, 1, 2, 3)}
            use_act = evac_i in act_sets[n_act]
            if use_act:
                nc.scalar.activation(
                    H_sb[:], H_ps[:], AF.Relu, bias=t.b1[:, hb:hb + 1]
                )
            else:
                nc.vector.tensor_scalar(
                    out=H_sb[:], in0=H_ps[:],
                    scalar1=t.b1[:, hb:hb + 1], scalar2=0.0,
                    op0=OP.add, op1=OP.max,
                )
            evac_i += 1
            H_sbs[(hb, jc)] = H_sb
    # sim = H @ W_attn2  (2-mm accumulation chains; keep each PSUM bank's
    # chains strictly sequential: group A completes before group B starts).
    # One 1-bank tile per jc half + per-jc exp (accum_out -> den) lets PE
    # run ahead of ACT.
    if _CONFIG.get("batch2"):
        EP = t.cur_EP2[:, half * N:(half + 1) * N]
    else:
        EP = wpool.tile([128, N], BF16, tag="EP", name="EP")
    if _CONFIG.get("split_sim", True):
        sim_tiles = [
            simpool.tile([128, 512], F32, tag="SIM_ps", name="SIM_ps")
            for _ in range(2)
        ]
    else:
        big = simpool.tile([128, N], F32, tag="SIM_ps", name="SIM_ps", bufs=1)
        sim_tiles = [big[:, 0:512], big[:, 512:1024]]
    for jc in range(2):
        SIM_ps = sim_tiles[jc]
        prev_last = None
        for g in range(2):
            insts = []
            for hb in range(2):
                inst = nc.tensor.matmul(
                    SIM_ps[g * DIM:(g + 1) * DIM, :],
                    t.W2[:, hb * DIM:(hb + 1) * DIM],
                    H_sbs[(hb, jc)][:, g * 512:(g + 1) * 512],
                    start=(hb == 0),
                    stop=(hb == 1),
                    tile_position=(0, g * DIM),
                )
                insts.append(inst)
            if prev_last is not None:
                add_dep_helper(
                    insts[0].ins, prev_last.ins, False,
                    "psum zero-region chain order",
                )
            prev_last = insts[1]
        if _CONFIG.get("split_sim", True):
            nc.scalar.activation(
                EP[:, jc * 512:(jc + 1) * 512], SIM_ps[:], AF.Exp,
                accum_out=t.dens[jc][:, m:m + 1],
            )
    if not _CONFIG.get("split_sim", True):
        nc.scalar.activation(
            EP[:, 0:N], big[:], AF.Exp, accum_out=t.dens[0][:, m:m + 1]
        )
    # v_e (mod q) = U + (v+k); P = E * V; then fold P pairwise at 2x and do
    # the final 1x tensor_reduce on only 256 elements per row. den comes from
    # the exp's accum_out, so the folds only process the product. With batch2
    # the whole tail runs once per TWO iterations at FD=2048.
    if _CONFIG.get("batch2"):
        if half == 1:
            V2 = wpool.tile([128, 2 * N], BF16, tag="V2", name="V2")
            nc.vector.tensor_tensor(
                out=V2[:], in0=t.cur_U2[:], in1=t.vk4[:], op=OP.add
            )
            SCR2 = wpool.tile([128, 2 * N], BF16, tag="SCR2", name="SCR2")
            nc.vector.tensor_tensor(
                out=SCR2[:], in0=t.cur_EP2[:], in1=V2[:], op=OP.mult
            )
            s3 = SCR2.rearrange("p (k n) -> p k n", k=2)
            F1 = wpool.tile([128, N], BF16, tag="F1", name="F1")
            f1 = F1.rearrange("p (k n) -> p k n", k=2)
            nc.vector.tensor_tensor(
                out=f1[:, :, :], in0=s3[:, :, 0:512], in1=s3[:, :, 512:1024],
                op=OP.add,
            )
            F2 = wpool.tile([128, N // 2], BF16, tag="F2", name="F2")
            f2 = F2.rearrange("p (k n) -> p k n", k=2)
            nc.vector.tensor_tensor(
                out=f2[:, :, :], in0=f1[:, :, 0:256], in1=f1[:, :, 256:512],
                op=OP.add,
            )
            F3 = wpool.tile([128, N // 4], BF16, tag="F3", name="F3")
            f3 = F3.rearrange("p (k n) -> p k n", k=2)
            nc.vector.tensor_tensor(
                out=f3[:, :, :], in0=f2[:, :, 0:128], in1=f2[:, :, 128:256],
                op=OP.add,
            )
            nc.vector.tensor_reduce(
                out=t.numU[:, m - 1:m + 1], in_=f3[:, :, :],
                axis=mybir.AxisListType.X, op=OP.add,
            )
    else:
        V_sb = wpool.tile([128, N], BF16, tag="V_sb", name="V_sb")
        nc.vector.tensor_tensor(out=V_sb[:], in0=U_sb[:], in1=t.vk2[:], op=OP.add)
        SCR1 = wpool.tile([128, N], BF16, tag="SCR1", name="SCR1")
        nc.vector.tensor_tensor(
            out=SCR1[:], in0=EP[:], in1=V_sb[:], op=OP.mult
        )
        F1 = wpool.tile([128, N // 2], BF16, tag="F1", name="F1")
        nc.vector.tensor_tensor(
            out=F1[:], in0=SCR1[:, 0:512], in1=SCR1[:, 512:1024], op=OP.add
        )
        F2 = wpool.tile([128, N // 4], BF16, tag="F2", name="F2")
        nc.vector.tensor_tensor(
            out=F2[:], in0=F1[:, 0:256], in1=F1[:, 256:512], op=OP.add
        )
        nc.vector.tensor_reduce(
            out=t.numU[:, m:m + 1], in_=F2[:],
            axis=mybir.AxisListType.X, op=OP.add,
        )


class _Tiles:
    pass


def _build_program(repeat=1):
    """Build the Bass program (same program for all 8 cores; per-core data
    comes from in_maps). Returns the Bass object. `repeat` re-runs the main
    loop N times inside the NEFF (for slope-based device timing)."""
    nc = bacc.Bacc("TRN2", debug=False, num_devices=1, target_bir_lowering=False)

    # ---- DRAM I/O ----
    # All inputs are packed host-side into two blobs (one per dtype) so the
    # kernel head issues 2 landing DMAs instead of 14.
    d_bP = nc.dram_tensor("blobP", [3, 1216], F32, kind="ExternalInput")
    d_bF = nc.dram_tensor("blobF", [128, 1348], F32, kind="ExternalInput")
    d_bB = nc.dram_tensor("blobB", [128, 512], BF16, kind="ExternalInput")
    d_out = nc.dram_tensor("agg_out", [128, NPAIR], F32, kind="ExternalOutput")

    with tile.TileContext(nc) as tc:
        with (
            tc.tile_pool(name="const", bufs=1) as cpool,
            tc.tile_pool(name="work", bufs=6) as wpool,
            tc.tile_pool(name="hsb", bufs=10) as hpool,
        ):
            t = _Tiles()
            # ---------------- persistent SBUF ----------------
            for name, shape, dt in (
                ("blobP", [3, 1216], F32),
                ("blobF", [128, 1348], F32), ("blobB", [128, 512], BF16),
                ("vk2", [128, N], BF16), ("vk4", [128, 2 * N], BF16),
                ("bn2", [128, N], BF16),
                ("aA", [DIM, NPAIR], F32), ("aB", [DIM, NPAIR], F32),
                ("qT2", [128, NPAIR], F32), ("qb2", [128, NPAIR], F32),
                ("den0", [128, NPAIR], F32), ("den1", [128, NPAIR], F32),
                ("numU", [128, NPAIR], F32),
                ("tmp_vT", [DIM, N], F32), ("kB", [DIM, N], BF16),
                ("warm", [128, 8], F32),
                ("recS0", [128, NPAIR], F32), ("agg", [128, NPAIR], F32),
            ):
                setattr(t, name, cpool.tile(shape, dt, tag=name, name=name))
            # blob slice views (same layout as _prep_inputs)
            bF, bB = t.blobF, t.blobB
            t.xT = bF[0:DIM, 0:1024]
            t.xTs = bF[0:DIM, 1024:1152]
            t.Wq = bF[0:DIM, 1152:1216]
            t.Wk = bF[0:DIM, 1216:1280]
            t.Wv = bF[0:DIM, 1280:1344]
            t.bpos1 = bF[0:DIM, 1344:1345]
            t.bpos2 = bF[0:DIM, 1345:1346]
            t.b1 = bF[:, 1346:1348]
            t.pT = t.blobP[:, 0:1024]
            t.pTs = t.blobP[:, 1024:1152]
            t.Wpos1 = t.blobP[:, 1152:1216]
            t.Wp2I = bB[:, 0:64]
            t.W1 = bB[:, 64:320]
            t.W2 = bB[:, 320:448]
            t.dens = [t.den0, t.den1]
            t.Rbufs = [
                (cpool.tile([128, N], BF16, tag=f"RA{p}", name=f"RA{p}"),
                 cpool.tile([128, N], BF16, tag=f"RB{p}", name=f"RB{p}"))
                for p in range(3)
            ]

            # ---------------- DMA loads ----------------
            # the small pos blob lands first: it gates the bn/a matmuls that
            # feed R(0) and hence the whole pipeline
            nc.sync.dma_start(t.blobP[:], d_bP.ap())
            nc.sync.dma_start(t.blobF[:], d_bF.ap())
            nc.sync.dma_start(t.blobB[:], d_bB.ap())

            # preload the exp table set early (one-time ~2.7us)
            nc.gpsimd.memset(t.warm[:], 0.0)
            nc.scalar.activation(t.warm[:], t.warm[:], AF.Exp)

            _setup_phase(nc, tc, t)
            if not _CONFIG.get("split_sim", True):
                nc.vector.memset(t.den1[:], 0.0)

            # ---------------- main loop over row pairs ----------------
            with (
                tc.tile_pool(name="u_ps", bufs=2, space="PSUM") as upool,
                tc.tile_pool(name="h_ps", bufs=2, space="PSUM") as hpspool,
                tc.tile_pool(name="s_ps", bufs=2, space="PSUM") as simpool,
            ):
                pools = (wpool, hpool, upool, hpspool, simpool)
                r_ahead = _CONFIG.get("r_ahead", 2)
                for _r in range(repeat):
                    for m in range(min(r_ahead, NPAIR)):
                        _emit_R(nc, t, m)
                    for m in range(NPAIR):
                        if m + r_ahead < NPAIR:
                            _emit_R(nc, t, m + r_ahead)
                        _pair_iteration(nc, t, pools, m)

            # ---------------- finalize ----------------
            nc.vector.tensor_tensor(
                out=t.den0[:], in0=t.den0[:], in1=t.den1[:], op=OP.add
            )
            nc.vector.reciprocal(t.recS0[:], t.den0[:])
            nc.vector.tensor_tensor(
                out=t.agg[:], in0=t.numU[:], in1=t.recS0[:], op=OP.mult
            )
            nc.vector.tensor_tensor(
                out=t.agg[:], in0=t.agg[:], in1=t.qT2[:], op=OP.subtract
            )
            nc.sync.dma_start(d_out.ap(), t.agg[:])

    nc.compile()
    return nc


def _prep_inputs(x, pos, W_qkv, W_pos1, b_pos1, W_pos2, b_pos2,
                 W_attn1, b_attn1, W_attn2, b_attn2):
    """Host-side data prep: slicing/transposes/weight packing (no O(N^2) math)."""
    bf = ml_dtypes.bfloat16
    x2 = np.ascontiguousarray(np.asarray(x, np.float32).reshape(N, DIM))
    p2 = np.ascontiguousarray(np.asarray(pos, np.float32).reshape(N, 3))
    xT = np.ascontiguousarray(x2.T)  # (64, N)
    pT = np.ascontiguousarray(p2.T)  # (3, N)
    W_qkv = np.asarray(W_qkv, np.float32)
    Wq = np.ascontiguousarray(W_qkv[:, 0:DIM])
    Wk = np.ascontiguousarray(W_qkv[:, DIM:2 * DIM])
    Wv = np.ascontiguousarray(W_qkv[:, 2 * DIM:3 * DIM])
    Wp2I = np.concatenate(
        [np.asarray(W_pos2, np.float32), -np.eye(DIM, dtype=np.float32)], axis=0
    ).astype(bf)  # (128, 64): [Wpos2; -I] for the combined U matmul
    W1dup = np.concatenate(
        [np.asarray(W_attn1, np.float32)] * 2, axis=0
    ).astype(bf)  # (128, 256)
    W2c = np.asarray(W_attn2, np.float32)
    W2cat = np.concatenate([W2c[0:128, :], W2c[128:256, :]], axis=1).astype(bf)
    b1c = np.ascontiguousarray(
        np.asarray(b_attn1, np.float32).reshape(2, 128).T
    )  # (128, 2)
    blobB = np.zeros((128, 512), dtype=bf)
    blobB[:, 0:64] = Wp2I
    blobB[:, 64:320] = W1dup
    blobB[:, 320:448] = W2cat

    blobF = np.zeros((128, 1348), dtype=np.float32)
    blobF[0:DIM, 0:1024] = xT
    blobF[0:DIM, 1152:1216] = Wq
    blobF[0:DIM, 1216:1280] = Wk
    blobF[0:DIM, 1280:1344] = Wv
    blobF[0:DIM, 1344:1345] = np.asarray(b_pos1, np.float32).reshape(DIM, 1)
    blobF[0:DIM, 1345:1346] = np.asarray(b_pos2, np.float32).reshape(DIM, 1)
    blobF[:, 1346:1348] = b1c

    in_maps = []
    for c in range(NCORES):
        bFc = blobF.copy()
        bFc[0:DIM, 1024:1152] = xT[:, c * ROWS:(c + 1) * ROWS]
        bPc = np.zeros((3, 1216), np.float32)
        bPc[:, 0:1024] = pT
        bPc[:, 1024:1152] = pT[:, c * ROWS:(c + 1) * ROWS]
        bPc[:, 1152:1216] = np.asarray(W_pos1, np.float32)
        in_maps.append({"blobP": bPc, "blobF": bFc, "blobB": blobB})
    return in_maps


def kernel(x, pos, W_qkv, W_pos1, b_pos1, W_pos2, b_pos2,
           W_attn1, b_attn1, W_attn2, b_attn2, _want_trace=False):
    if "nc" not in _CACHE:
        _CACHE["nc"] = _build_program()
    nc = _CACHE["nc"]
    in_maps = _prep_inputs(x, pos, W_qkv, W_pos1, b_pos1, W_pos2, b_pos2,
                           W_attn1, b_attn1, W_attn2, b_attn2)
    res = run_bass_kernel_spmd(
        nc, in_maps, core_ids=list(range(NCORES)), trace=_want_trace
    )
    _CACHE["last_results"] = res
    out = np.empty((N, DIM), np.float32)
    for c in range(NCORES):
        agg = np.asarray(res.results[c]["agg_out"], np.float32)  # (128, 64)
        out[c * ROWS:c * ROWS + NPAIR, :] = agg[0:DIM, :].T
        out[c * ROWS + NPAIR:(c + 1) * ROWS, :] = agg[DIM:128, :].T
    return out.reshape(1, N, DIM)
